# revision 8
# baseline (speedup 1.0000x reference)
# Trainium2 Bass kernel: nn_DecoderAttentionLayer (sliding-window decoder layer)
# Sequence-parallel over 8 NeuronCores: core = (n, quarter); each core processes
# 1024 tokens (+128-token halo for the previous key/value chunk).
#
# v2 design notes (vs 556us baseline):
#   - QKV and o_proj matmuls in fp8e4 DoubleRow mode (2 k-tiles per
#     instruction, 0.5 cycles/row) with weights pre-scaled on host
#     (w*64, ow*4096) so fp8 stays in normal range; scales folded back
#     into the rms/evac constants.
#   - q normalized on-chip before store (exp scale becomes the constant 1.0).
#   - mask applied ADDITIVELY pre-exp via eye@mask matmul accumulation
#     (kills the gpsimd multiplicative-mask pass).
#   - softmax denominator from the EXP instruction's accum_out (kills the
#     DVE reduce); probs normalized during the PE transpose by using
#     diag(1/den) instead of the identity (kills the DVE normalize pass).
#   - attn-out and o_proj contract in fp8 DoubleRow.
#   - phases A (qkv+prep), B (attention), C (o_proj) interleaved per chunk;
#     per-pair software pipeline keeps PE fed.
#   - x kept resident in SBUF for the residual (no phase-C reload);
#     compact rotary table [TLOC, 64] instead of [TLOC, 1024].
import sys
import numpy as np
import ml_dtypes

sys.path.insert(0, "/opt/trn_rl_repo")

import bass_rust
import concourse.bass as bass
import concourse.tile as tile
from concourse import mybir
from concourse.bass_utils import run_bass_kernel_spmd
from concourse.vector_clock import ScopedClock

F32 = mybir.dt.float32
BF16 = mybir.dt.bfloat16
F8 = mybir.dt.float8e4
AF = mybir.ActivationFunctionType
ALU = mybir.AluOpType
DR = mybir.MatmulPerfMode.DoubleRow
BF = ml_dtypes.bfloat16
E4 = ml_dtypes.float8_e4m3

N, T, D = 2, 4096, 1024
HD, NH, W = 64, 16, 128
EPS = 1.1920929e-07
TLOC = 1152          # 128 halo + 1024 own tokens
NCH = 9              # x chunks per core (chunk 0 = halo)
NPAIR = 8            # head pairs


# ---------------------------------------------------------------------------
# Compiler workarounds: walrus in this container accepts at most ONE sem wait
# per instruction on most structs. Split excess waits onto NoOps.
# ---------------------------------------------------------------------------
def _split_excess_waits(nc):
    cnt = 0
    for f in nc.m.functions:
        for b in f.blocks:
            changed = False
            new_insts = []
            for inst in b.instructions:
                si = inst.sync_info
                waits = list(si.on_wait) if (si is not None and si.on_wait) else []
                if len(waits) > 1:
                    si.on_wait = waits[:1]
                    for w in waits[1:]:
                        cnt += 1
                        nop = bass_rust.InstNoOp(
                            name=f"I-waitfix-{cnt}", engine=inst.engine)
                        nop.sync_info = mybir.SyncInfo(on_wait=[w], on_update=[])
                        new_insts.append(nop)
                    changed = True
                new_insts.append(inst)
            if changed:
                b.instructions = new_insts
    return cnt


def _patched_drain_and_barrier(self, tick_clock, wait_clock):
    drain_inst = self.nc.sync.drain()
    wait_clock.add_sem_waits(
        drain_inst.ins, ScopedClock({None: tick_clock.global_clock}))
    si = drain_inst.ins.sync_info
    if si is not None and si.on_wait and len(si.on_wait) > 1:
        waits = list(si.on_wait)
        si.on_wait = waits[:1]
        for w in waits[1:]:
            extra = self.nc.sync.drain()
            esi = extra.ins.sync_info
            if esi is None:
                extra.ins.sync_info = mybir.SyncInfo(on_wait=[w], on_update=[])
            else:
                esi.on_wait = [w]
    self.nc.all_engine_barrier()
    assert self.sems is not None
    popped = self.nc._tile_sem_poison_stack.pop()
    assert popped is self._sem_poison
    self.nc.clear_and_free_semaphores(list(self.sems.allocated().values()))
    self.nc.all_engine_barrier()


tile.TileContext._drain_and_barrier = _patched_drain_and_barrier


def _ap(t, offset, dims):
    return bass.AP(tensor=t.tensor, offset=t.offset + offset, ap=[t.ap[0]] + dims)


def build_program(waitfix=True):
    nc = bass.Bass()

    x_nat = nc.dram_tensor("x_nat", [TLOC, D], F32, kind="ExternalInput")
    xT8d = nc.dram_tensor("xT8", [128, 8, TLOC], F8, kind="ExternalInput")
    wT8d = nc.dram_tensor("wT8", [128, 8, 3 * D], F8, kind="ExternalInput")
    ow8d = nc.dram_tensor("ow8", [128, 8, D], F8, kind="ExternalInput")
    rotd = nc.dram_tensor("rotc", [TLOC, 64], BF16, kind="ExternalInput")
    mFd = nc.dram_tensor("maskF", [W, 2 * W], F8, kind="ExternalInput")
    mRd = nc.dram_tensor("maskR", [W, 2 * W], F8, kind="ExternalInput")
    eyebd = nc.dram_tensor("eyeb", [128, 128], BF16, kind="ExternalInput")
    eye8d = nc.dram_tensor("eye8", [128, 128], F8, kind="ExternalInput")
    y = nc.dram_tensor("y", [1024, D], F32, kind="ExternalOutput")

    with tile.TileContext(nc) as tc:
        with tc.tile_pool(name="persist", bufs=1) as P, \
             tc.tile_pool(name="qkwork", bufs=3) as QK, \
             tc.tile_pool(name="small", bufs=4) as SM, \
             tc.tile_pool(name="pbwork", bufs=3) as PB, \
             tc.tile_pool(name="ypool", bufs=2) as YP, \
             tc.tile_pool(name="ps_a", bufs=2, space="PSUM") as PSA, \
             tc.tile_pool(name="ps_t", bufs=2, space="PSUM") as PST:

            # ---------------- persistent loads (order matters for startup) --
            # first chunk's inputs first so PE can start ASAP
            x_sb = []
            for c in range(NCH):
                x_sb.append(P.tile([128, D], F32, tag=f"x{c}", name=f"x{c}"))
            xc8 = []
            for c in range(NCH):
                xc8.append(P.tile([128, 8, 128], F8, tag=f"xc{c}", name=f"xc{c}"))
            wT4 = []
            for kp in range(4):
                wT4.append(P.tile([128, 2, 3 * D], F8, tag=f"w{kp}", name=f"w{kp}"))
            rc_t = []
            for c in range(NCH):
                rc_t.append(P.tile([128, 64], BF16, tag=f"rc{c}", name=f"rc{c}"))

            nc.sync.dma_start(out=x_sb[0], in_=x_nat[0:128, :])
            nc.sync.dma_start(out=xc8[0], in_=xT8d[:, :, 0:128])
            for kp in range(4):
                nc.sync.dma_start(out=wT4[kp], in_=wT8d[:, 2 * kp:2 * kp + 2, :])
            eye_b = P.tile([128, 128], BF16, tag="eyeb")
            nc.sync.dma_start(out=eye_b, in_=eyebd[:, :])
            eye_8 = P.tile([128, 128], F8, tag="eye8")
            nc.sync.dma_start(out=eye_8, in_=eye8d[:, :])
            mF = P.tile([W, 2 * W], F8, tag="mF")
            nc.sync.dma_start(out=mF, in_=mFd[:, :])
            mR = P.tile([W, 2 * W], F8, tag="mR")
            nc.sync.dma_start(out=mR, in_=mRd[:, :])
            nc.scalar.dma_start(out=rc_t[0], in_=rotd[0:128, :])
            for c in range(1, NCH):
                nc.sync.dma_start(out=xc8[c], in_=xT8d[:, :, c * 128:(c + 1) * 128])
                nc.scalar.dma_start(out=x_sb[c], in_=x_nat[c * 128:(c + 1) * 128, :])
                nc.scalar.dma_start(out=rc_t[c], in_=rotd[c * 128:(c + 1) * 128, :])
            ow8 = P.tile([128, 8, D], F8, tag="ow8")
            nc.sync.dma_start(out=ow8, in_=ow8d[:, :, :])

            eps_t = P.tile([128, 1], F32, tag="eps")
            nc.vector.memset(eps_t, EPS)

            # persistent activation stores
            qT8 = P.tile([128, NPAIR * TLOC], F8, tag="qT8")
            kT8 = P.tile([128, NPAIR * TLOC], F8, tag="kT8")
            v8 = P.tile([128, NCH * D], F8, tag="v8")
            at8 = P.tile([128, NPAIR * 1024], F8, tag="at8")

            # ---------------- phase A for one chunk -------------------------
            def phase_a(c):
                # rms stats for x chunk
                bstats = SM.tile([128, 2, 6], F32, tag="bstats")
                for g in range(2):
                    nc.vector.bn_stats(out=bstats[:, g, :],
                                       in_=x_sb[c][:, g * 512:(g + 1) * 512])
                mv = SM.tile([128, 2], F32, tag="mv")
                nc.vector.bn_aggr(out=mv, in_=bstats)
                msq = SM.tile([128, 1], F32, tag="msq")
                nc.vector.tensor_mul(msq, mv[:, 0:1], mv[:, 0:1])
                nc.vector.tensor_add(msq, msq, mv[:, 1:2])
                rsq = SM.tile([128, 1], F32, tag="rsq")
                nc.scalar.activation(out=rsq, in_=msq, func=AF.Sqrt, bias=eps_t)
                inv = SM.tile([128, 1], F32, tag="inv")
                nc.vector.reciprocal(out=inv, in_=rsq)
                inv64 = SM.tile([128, 1], F32, tag="inv64")
                nc.vector.tensor_scalar_mul(out=inv64, in0=inv, scalar1=1.0 / 64.0)
                inv2 = SM.tile([128, 1], F32, tag="inv2")
                nc.vector.tensor_mul(inv2, inv, inv)

                for half in range(2):
                    ps = PSA.tile([128, 3, 512], F32, tag="qkv")
                    for kp in range(4):
                        lhs = _ap(xc8[c], 2 * kp * 128, [[128, 2], [1, 128]])
                        for slot, jlo in ((0, 0), (1, D), (2, 2 * D)):
                            nc.tensor.matmul(
                                ps[:, slot, :], lhs,
                                _ap(wT4[kp], jlo + half * 512,
                                    [[3 * D, 2], [1, 512]]),
                                start=(kp == 0), stop=(kp == 3), perf_mode=DR)
                    # V evac: fp8 with inv/64 folded
                    nc.scalar.activation(
                        out=v8[:, c * D + half * 512: c * D + (half + 1) * 512],
                        in_=ps[:, 2, :], func=AF.Copy, scale=inv64)
                    tp = PST.tile([128, 1024], BF16, tag="tp")
                    for which, slot in (("q", 0), ("k", 1)):
                        if which == "q" and c == 0:
                            continue
                        sq = QK.tile([128, 512], BF16, tag="sq")
                        nc.scalar.activation(out=sq, in_=ps[:, slot, :],
                                             func=AF.Square)
                        ssq = SM.tile([128, 8], F32, tag="ssq")
                        nc.vector.tensor_reduce(
                            out=ssq, in_=sq.rearrange("p (h d) -> p h d", h=8),
                            axis=mybir.AxisListType.X, op=ALU.add)
                        mt = SM.tile([128, 8], F32, tag="mt")
                        nc.vector.tensor_scalar(
                            out=mt, in0=ssq, scalar1=inv2,
                            scalar2=1.0 / (4096.0 * 64.0),
                            op0=ALU.mult, op1=ALU.mult)
                        rs = SM.tile([128, 8], F32, tag="rs")
                        nc.scalar.activation(out=rs, in_=mt, func=AF.Sqrt,
                                             bias=eps_t)
                        rr = SM.tile([128, 8], F32, tag="rr")
                        nc.vector.reciprocal(out=rr, in_=rs)
                        scl = SM.tile([128, 8], F32, tag="scl")
                        nc.vector.tensor_scalar(
                            out=scl, in0=rr, scalar1=inv,
                            scalar2=(1.0 / 512.0 if which == "q" else 1.0 / 64.0),
                            op0=ALU.mult, op1=ALU.mult)
                        qn = QK.tile([128, 512], BF16, tag=f"{which}n")
                        nc.vector.tensor_mul(
                            qn.rearrange("p (h d) -> p h d", h=8),
                            ps[:, slot, :].rearrange("p (h d) -> p h d", h=8),
                            _ap(scl, 0, [[1, 8], [0, HD]]))
                        # rotary on the active 16-col blocks
                        qs = QK.tile([128, 8, 2, 16], BF16, tag="qs")
                        nc.vector.tensor_copy(
                            qs, _ap(qn, 32, [[64, 8], [-32, 2], [1, 16]]))
                        t1 = QK.tile([128, 8, 2, 16], BF16, tag="t1")
                        nc.vector.tensor_mul(
                            t1, qs, _ap(rc_t[c], 32, [[0, 8], [16, 2], [1, 16]]))
                        act = _ap(qn, 0, [[64, 8], [32, 2], [1, 16]])
                        nc.gpsimd.tensor_mul(
                            act, act, _ap(rc_t[c], 0, [[0, 8], [16, 2], [1, 16]]))
                        nc.gpsimd.tensor_add(act, act, t1)
                        # transpose 4 pair-blocks
                        tpo = 0 if which == "q" else 512
                        for i in range(4):
                            nc.tensor.transpose(
                                tp[:, tpo + i * 128: tpo + (i + 1) * 128],
                                qn[:, i * 128:(i + 1) * 128], eye_b)
                        dst = qT8 if which == "q" else kT8
                        nc.scalar.copy(
                            _ap(dst, (half * 4) * TLOC + c * 128,
                                [[TLOC, 4], [1, 128]]),
                            tp[:, tpo:tpo + 512])

            # ---------------- phase B stages (per chunk c, pair p) ----------
            def stage_s(c, p):
                """mask+scores matmuls, exp with accum den, invd, diag."""
                mask = mF if c == 1 else mR
                pt = PSA.tile([128, 3, 512], F32, tag="qkv")
                for hh in range(2):
                    off = p * TLOC
                    nc.tensor.matmul(
                        pt[:, hh, 0:256], eye_8, mask[:, :],
                        start=True, stop=False)
                    nc.tensor.matmul(
                        pt[:, hh, 0:256],
                        qT8[hh * 64:(hh + 1) * 64,
                            off + c * 128: off + (c + 1) * 128],
                        kT8[hh * 64:(hh + 1) * 64,
                            off + (c - 1) * 128: off + (c + 1) * 128],
                        start=False, stop=True)
                e_sb = PB.tile([128, 512], BF16, tag="e_sb")
                den = PB.tile([128, 2], F32, tag="den")
                for hh in range(2):
                    nc.scalar.activation(
                        out=e_sb[:, hh * 256:(hh + 1) * 256],
                        in_=pt[:, hh, 0:256], func=AF.Exp,
                        accum_out=den[:, hh:hh + 1])
                invd = PB.tile([128, 2], F32, tag="invd")
                nc.vector.reciprocal(out=invd, in_=den)
                diag = PB.tile([128, 256], BF16, tag="diag")
                for hh in range(2):
                    nc.vector.tensor_scalar_mul(
                        out=diag[:, hh * 128:(hh + 1) * 128],
                        in0=eye_b, scalar1=invd[:, hh:hh + 1])
                return pt, e_sb, diag

            def stage_t(c, p, st):
                """probs transpose-normalize into psum, copy to fp8 sbuf."""
                pt, e_sb, diag = st
                for i in range(4):
                    nc.tensor.matmul(
                        pt[:, 2, i * 128:(i + 1) * 128],
                        e_sb[:, i * 128:(i + 1) * 128],
                        diag[:, (i // 2) * 128:(i // 2 + 1) * 128],
                        start=True, stop=True)
                pT8 = PB.tile([128, 512], F8, tag="pT8")
                nc.vector.tensor_copy(pT8, pt[:, 2, :])
                return pt, pT8

            def stage_v(c, p, st):
                """attn-out DoubleRow matmuls + fp8 store."""
                pt, pT8 = st
                for hh in range(2):
                    for kc in range(2):
                        nc.tensor.matmul(
                            pt[hh * 64:(hh + 1) * 64, 0, 384:512],
                            v8[:, (c - 1 + kc) * D + (2 * p + hh) * HD:
                               (c - 1 + kc) * D + (2 * p + hh + 1) * HD],
                            pT8[:, (hh * 2 + kc) * 128:(hh * 2 + kc + 1) * 128],
                            start=(kc == 0), stop=(kc == 1),
                            tile_position=(0, hh * 64))
                nc.scalar.copy(
                    at8[:, p * 1024 + (c - 1) * 128: p * 1024 + c * 128],
                    pt[:, 0, 384:512])

            # ---------------- phase B+C for one chunk -----------------------
            def phase_bc(c):
                st_s = {}
                st_t = {}
                for p in range(NPAIR + 2):
                    if p < NPAIR:
                        st_s[p] = stage_s(c, p)
                    if 1 <= p <= NPAIR:
                        st_t[p - 1] = stage_t(c, p - 1, st_s.pop(p - 1))
                    if p >= 2:
                        stage_v(c, p - 2, st_t.pop(p - 2))
                # o_proj + residual
                o_ps = PSA.tile([128, 3, 512], F32, tag="qkv")
                for half in range(2):
                    for kp in range(4):
                        nc.tensor.matmul(
                            o_ps[:, half, :],
                            _ap(at8, 2 * kp * 1024 + (c - 1) * 128,
                                [[1024, 2], [1, 128]]),
                            _ap(ow8, 2 * kp * D + half * 512,
                                [[D, 2], [1, 512]]),
                            start=(kp == 0), stop=(kp == 3), perf_mode=DR)
                for half in range(2):
                    yt = YP.tile([128, 512], F32, tag="y")
                    nc.vector.scalar_tensor_tensor(
                        out=yt, in0=o_ps[:, half, :], scalar=1.0 / 4096.0,
                        in1=x_sb[c][:, half * 512:(half + 1) * 512],
                        op0=ALU.mult, op1=ALU.add)
                    nc.scalar.dma_start(
                        out=y[(c - 1) * 128:c * 128,
                              half * 512:(half + 1) * 512], in_=yt)

            # ---------------- interleaved schedule --------------------------
            phase_a(0)
            for c in range(1, NCH):
                phase_a(c)
                phase_bc(c)

    if waitfix:
        _split_excess_waits(nc)
    return nc


_PROGRAM = None


def _get_program():
    global _PROGRAM
    if _PROGRAM is None:
        _PROGRAM = build_program()
    return _PROGRAM


def _q8(a):
    return np.clip(a, -240.0, 240.0).astype(E4)


def _host_inputs(input_NTD, qkv_weight, o_weight, o_scale):
    x = np.asarray(input_NTD, dtype=np.float32)
    wq = np.asarray(qkv_weight, dtype=np.float32).reshape(3 * D, D)
    # [128, 8, 3D]: wT8[p, kt, j] = wq[j, kt*128+p] * 64
    wT8 = _q8(np.ascontiguousarray(
        (wq.T * 64.0).reshape(8, 128, 3 * D).transpose(1, 0, 2)))
    ows = np.asarray(o_weight, dtype=np.float32) * \
        np.asarray(o_scale, dtype=np.float32)[:, None]
    ow8 = _q8(np.ascontiguousarray(
        (ows.T * 4096.0).reshape(8, 128, D).transpose(1, 0, 2)))
    eyeb = np.eye(128, dtype=np.float32).astype(BF)
    eye8 = np.eye(128, dtype=np.float32).astype(E4)

    j = np.arange(W)[:, None]
    m = np.arange(2 * W)[None, :]
    base = (m > j) & (m <= W + j)
    maskR = np.where(base, 0.0, -240.0).astype(np.float32).astype(E4)
    maskF0 = np.where(base & (m >= W), 0.0, -240.0).astype(np.float32).astype(E4)

    freqs = (1.0 / 10000.0) ** np.linspace(0.0, 1.0, 16).astype(np.float32)

    in_maps = []
    for core in range(8):
        n, qq = divmod(core, 4)
        lo = qq * 1024 - 128
        if qq == 0:
            xs = np.concatenate(
                [np.zeros((128, D), np.float32), x[n, 0:1024]], axis=0)
        else:
            xs = x[n, lo:lo + 1024 + 128]
        xs = np.ascontiguousarray(xs)
        xT8 = _q8(np.ascontiguousarray(
            xs.T.reshape(8, 128, TLOC).transpose(1, 0, 2)))
        pos = np.maximum(np.arange(lo, lo + TLOC), 0).astype(np.float32)
        theta = pos[:, None] * freqs[None, :]
        cos16, sin16 = np.cos(theta), np.sin(theta)
        rotc = np.ascontiguousarray(np.concatenate(
            [cos16, cos16, sin16, -sin16], axis=1)).astype(BF)
        in_maps.append(dict(
            x_nat=xs, xT8=xT8, wT8=wT8, ow8=ow8, rotc=rotc,
            maskF=(maskF0 if qq == 0 else maskR), maskR=maskR,
            eyeb=eyeb, eye8=eye8))
    return in_maps


def kernel(input_NTD, qkv_weight, o_weight, o_scale, _trace=False):
    nc = _get_program()
    in_maps = _host_inputs(input_NTD, qkv_weight, o_weight, o_scale)
    res = run_bass_kernel_spmd(nc, in_maps, core_ids=list(range(8)),
                               trace=_trace)
    kernel.last_results = res
    out = np.empty((N, T, D), dtype=np.float32)
    for core in range(8):
        n, qq = divmod(core, 4)
        out[n, qq * 1024:(qq + 1) * 1024] = res.results[core]["y"]
    return out


# revision 10
# speedup vs baseline: 1.2894x; 1.2894x over previous
# Trainium2 Bass kernel: nn_DecoderAttentionLayer (sliding-window decoder layer)
# Sequence-parallel over 8 NeuronCores: core = (n, quarter); each core processes
# 1024 tokens (+128-token halo for the previous key/value chunk).
#
# v3 design notes:
#   - QKV and o_proj matmuls in fp8e4 DoubleRow mode (2 k-tiles per
#     instruction, 0.5 cycles/row) with weights pre-scaled on host
#     (w*64, ow*4096); scales folded into the rms/evac constants.
#   - q normalized on-chip before store (exp scale is the constant 1.0).
#   - mask applied ADDITIVELY pre-exp via eye@mask matmul accumulation.
#   - one merged EXP per pair (strided psum read), denominator via one DVE
#     reduce, probs normalized in-place on DVE before the PE transpose.
#   - attention probs/values kept bf16; attn output stored fp8 for the
#     DoubleRow o_proj.
#   - phases A (qkv+prep), B (attention), C (o_proj) interleaved per chunk;
#     per-pair software pipeline (S, T at -2, V at -3) keeps PE fed.
#   - x kept resident in SBUF for the residual; compact rotary table.
#   - all DMA issues on the sync queue (keeps compute-queue dispatch clean).
import sys
import numpy as np
import ml_dtypes

sys.path.insert(0, "/opt/trn_rl_repo")

import bass_rust
import concourse.bass as bass
import concourse.tile as tile
from concourse import mybir
from concourse.bass_utils import run_bass_kernel_spmd
from concourse.vector_clock import ScopedClock

F32 = mybir.dt.float32
BF16 = mybir.dt.bfloat16
F8 = mybir.dt.float8e4
AF = mybir.ActivationFunctionType
ALU = mybir.AluOpType
DR = mybir.MatmulPerfMode.DoubleRow
BF = ml_dtypes.bfloat16
E4 = ml_dtypes.float8_e4m3

N, T, D = 2, 4096, 1024
HD, NH, W = 64, 16, 128
EPS = 1.1920929e-07
TLOC = 1152          # 128 halo + 1024 own tokens
NCH = 9              # x chunks per core (chunk 0 = halo)
NPAIR = 8            # head pairs


# ---------------------------------------------------------------------------
# Compiler workarounds: walrus in this container accepts at most ONE sem wait
# per instruction on most structs. Split excess waits onto NoOps.
# ---------------------------------------------------------------------------
def _split_excess_waits(nc):
    cnt = 0
    for f in nc.m.functions:
        for b in f.blocks:
            changed = False
            new_insts = []
            for inst in b.instructions:
                si = inst.sync_info
                waits = list(si.on_wait) if (si is not None and si.on_wait) else []
                if len(waits) > 1:
                    si.on_wait = waits[:1]
                    for w in waits[1:]:
                        cnt += 1
                        nop = bass_rust.InstNoOp(
                            name=f"I-waitfix-{cnt}", engine=inst.engine)
                        nop.sync_info = mybir.SyncInfo(on_wait=[w], on_update=[])
                        new_insts.append(nop)
                    changed = True
                new_insts.append(inst)
            if changed:
                b.instructions = new_insts
    return cnt


def _patched_drain_and_barrier(self, tick_clock, wait_clock):
    drain_inst = self.nc.sync.drain()
    wait_clock.add_sem_waits(
        drain_inst.ins, ScopedClock({None: tick_clock.global_clock}))
    si = drain_inst.ins.sync_info
    if si is not None and si.on_wait and len(si.on_wait) > 1:
        waits = list(si.on_wait)
        si.on_wait = waits[:1]
        for w in waits[1:]:
            extra = self.nc.sync.drain()
            esi = extra.ins.sync_info
            if esi is None:
                extra.ins.sync_info = mybir.SyncInfo(on_wait=[w], on_update=[])
            else:
                esi.on_wait = [w]
    self.nc.all_engine_barrier()
    assert self.sems is not None
    popped = self.nc._tile_sem_poison_stack.pop()
    assert popped is self._sem_poison
    self.nc.clear_and_free_semaphores(list(self.sems.allocated().values()))
    self.nc.all_engine_barrier()


tile.TileContext._drain_and_barrier = _patched_drain_and_barrier


def _ap(t, offset, dims):
    return bass.AP(tensor=t.tensor, offset=t.offset + offset, ap=[t.ap[0]] + dims)


def build_program(waitfix=True):
    nc = bass.Bass()

    x_nat = nc.dram_tensor("x_nat", [TLOC, D], F32, kind="ExternalInput")
    xT8d = nc.dram_tensor("xT8", [128, 8, TLOC], F8, kind="ExternalInput")
    wT8d = nc.dram_tensor("wT8", [128, 8, 3 * D], F8, kind="ExternalInput")
    ow8d = nc.dram_tensor("ow8", [128, 8, D], F8, kind="ExternalInput")
    rotd = nc.dram_tensor("rotc", [TLOC, 64], BF16, kind="ExternalInput")
    mFd = nc.dram_tensor("maskF", [W, 2 * W], F8, kind="ExternalInput")
    mRd = nc.dram_tensor("maskR", [W, 2 * W], F8, kind="ExternalInput")
    eyebd = nc.dram_tensor("eyeb", [128, 128], BF16, kind="ExternalInput")
    eye8d = nc.dram_tensor("eye8", [128, 128], F8, kind="ExternalInput")
    y = nc.dram_tensor("y", [1024, D], F32, kind="ExternalOutput")

    with tile.TileContext(nc) as tc:
        with tc.tile_pool(name="persist", bufs=1) as P, \
             tc.tile_pool(name="qkwork", bufs=3) as QK, \
             tc.tile_pool(name="small", bufs=4) as SM, \
             tc.tile_pool(name="pbwork", bufs=4) as PB, \
             tc.tile_pool(name="ypool", bufs=2) as YP, \
             tc.tile_pool(name="ps_a", bufs=2, space="PSUM") as PSA, \
             tc.tile_pool(name="ps_v", bufs=2, space="PSUM") as PSV, \
             tc.tile_pool(name="ps_t", bufs=2, space="PSUM") as PST:

            # ---------------- persistent loads (order matters for startup) --
            x_sb = []
            for c in range(NCH):
                x_sb.append(P.tile([128, D], F32, tag=f"x{c}", name=f"x{c}"))
            xc8 = []
            for c in range(NCH):
                xc8.append(P.tile([128, 8, 128], F8, tag=f"xc{c}", name=f"xc{c}"))
            wT4 = []
            for kp in range(4):
                wT4.append(P.tile([128, 2, 3 * D], F8, tag=f"w{kp}", name=f"w{kp}"))
            rc_t = []
            for c in range(NCH):
                rc_t.append(P.tile([128, 64], BF16, tag=f"rc{c}", name=f"rc{c}"))

            nc.sync.dma_start(out=x_sb[0], in_=x_nat[0:128, :])
            nc.sync.dma_start(out=xc8[0], in_=xT8d[:, :, 0:128])
            for kp in range(4):
                nc.sync.dma_start(out=wT4[kp], in_=wT8d[:, 2 * kp:2 * kp + 2, :])
            eye_b = P.tile([128, 128], BF16, tag="eyeb")
            nc.sync.dma_start(out=eye_b, in_=eyebd[:, :])
            eye_8 = P.tile([128, 128], F8, tag="eye8")
            nc.sync.dma_start(out=eye_8, in_=eye8d[:, :])
            mF = P.tile([W, 2 * W], F8, tag="mF")
            nc.sync.dma_start(out=mF, in_=mFd[:, :])
            mR = P.tile([W, 2 * W], F8, tag="mR")
            nc.sync.dma_start(out=mR, in_=mRd[:, :])
            nc.sync.dma_start(out=rc_t[0], in_=rotd[0:128, :])
            for c in range(1, NCH):
                nc.sync.dma_start(out=xc8[c], in_=xT8d[:, :, c * 128:(c + 1) * 128])
                nc.sync.dma_start(out=x_sb[c], in_=x_nat[c * 128:(c + 1) * 128, :])
                nc.sync.dma_start(out=rc_t[c], in_=rotd[c * 128:(c + 1) * 128, :])
            ow8 = P.tile([128, 8, D], F8, tag="ow8")
            nc.sync.dma_start(out=ow8, in_=ow8d[:, :, :])

            eps_t = P.tile([128, 1], F32, tag="eps")
            nc.vector.memset(eps_t, EPS)

            # persistent activation stores
            qT8 = P.tile([128, NPAIR * TLOC], F8, tag="qT8")
            kT8 = P.tile([128, NPAIR * TLOC], F8, tag="kT8")
            v_bf = P.tile([128, NCH * D], BF16, tag="v_bf")
            at8 = P.tile([128, NPAIR * 1024], F8, tag="at8")

            # ---------------- phase A for one chunk -------------------------
            def phase_a(c):
                # rms stats for x chunk
                bstats = SM.tile([128, 2, 6], F32, tag="bstats")
                for g in range(2):
                    nc.vector.bn_stats(out=bstats[:, g, :],
                                       in_=x_sb[c][:, g * 512:(g + 1) * 512])
                mv = SM.tile([128, 2], F32, tag="mv")
                nc.vector.bn_aggr(out=mv, in_=bstats)
                msq = SM.tile([128, 1], F32, tag="msq")
                nc.vector.tensor_mul(msq, mv[:, 0:1], mv[:, 0:1])
                nc.vector.tensor_add(msq, msq, mv[:, 1:2])
                rsq = SM.tile([128, 1], F32, tag="rsq")
                nc.scalar.activation(out=rsq, in_=msq, func=AF.Sqrt, bias=eps_t)
                inv = SM.tile([128, 1], F32, tag="inv")
                nc.vector.reciprocal(out=inv, in_=rsq)
                inv64 = SM.tile([128, 1], F32, tag="inv64")
                nc.vector.tensor_scalar_mul(out=inv64, in0=inv, scalar1=1.0 / 64.0)
                inv2 = SM.tile([128, 1], F32, tag="inv2")
                nc.vector.tensor_mul(inv2, inv, inv)

                for half in range(2):
                    ps = PSA.tile([128, 2, 512], F32, tag="qkv")
                    for kp in range(4):
                        lhs = _ap(xc8[c], 2 * kp * 128, [[128, 2], [1, 128]])
                        for slot, jlo in ((0, 0), (1, D)):
                            nc.tensor.matmul(
                                ps[:, slot, :], lhs,
                                _ap(wT4[kp], jlo + half * 512,
                                    [[3 * D, 2], [1, 512]]),
                                start=(kp == 0), stop=(kp == 3), perf_mode=DR)
                    vps = PSV.tile([128, 512], F32, tag="sv")
                    for kp in range(4):
                        nc.tensor.matmul(
                            vps, _ap(xc8[c], 2 * kp * 128, [[128, 2], [1, 128]]),
                            _ap(wT4[kp], 2 * D + half * 512,
                                [[3 * D, 2], [1, 512]]),
                            start=(kp == 0), stop=(kp == 3), perf_mode=DR)
                    # V evac: bf16 with inv/64 folded
                    nc.scalar.activation(
                        out=v_bf[:, c * D + half * 512: c * D + (half + 1) * 512],
                        in_=vps, func=AF.Copy, scale=inv64)
                    tp = PST.tile([128, 1024], BF16, tag="tp")
                    for which, slot in (("q", 0), ("k", 1)):
                        if which == "q" and c == 0:
                            continue
                        sq = QK.tile([128, 512], BF16, tag="sq")
                        nc.scalar.activation(out=sq, in_=ps[:, slot, :],
                                             func=AF.Square)
                        ssq = SM.tile([128, 8], F32, tag="ssq")
                        nc.vector.tensor_reduce(
                            out=ssq, in_=sq.rearrange("p (h d) -> p h d", h=8),
                            axis=mybir.AxisListType.X, op=ALU.add)
                        mt = SM.tile([128, 8], F32, tag="mt")
                        nc.vector.tensor_scalar(
                            out=mt, in0=ssq, scalar1=inv2,
                            scalar2=1.0 / (4096.0 * 64.0),
                            op0=ALU.mult, op1=ALU.mult)
                        rs = SM.tile([128, 8], F32, tag="rs")
                        nc.scalar.activation(out=rs, in_=mt, func=AF.Sqrt,
                                             bias=eps_t)
                        rr = SM.tile([128, 8], F32, tag="rr")
                        nc.vector.reciprocal(out=rr, in_=rs)
                        scl = SM.tile([128, 8], F32, tag="scl")
                        nc.vector.tensor_scalar(
                            out=scl, in0=rr, scalar1=inv,
                            scalar2=(1.0 / 512.0 if which == "q" else 1.0 / 64.0),
                            op0=ALU.mult, op1=ALU.mult)
                        qn = QK.tile([128, 512], BF16, tag=f"{which}n")
                        nc.vector.tensor_mul(
                            qn.rearrange("p (h d) -> p h d", h=8),
                            ps[:, slot, :].rearrange("p (h d) -> p h d", h=8),
                            _ap(scl, 0, [[1, 8], [0, HD]]))
                        # rotary on the active 16-col blocks
                        qs = QK.tile([128, 8, 2, 16], BF16, tag="qs")
                        nc.vector.tensor_copy(
                            qs, _ap(qn, 32, [[64, 8], [-32, 2], [1, 16]]))
                        t1 = QK.tile([128, 8, 2, 16], BF16, tag="t1")
                        nc.vector.tensor_mul(
                            t1, qs, _ap(rc_t[c], 32, [[0, 8], [16, 2], [1, 16]]))
                        act = _ap(qn, 0, [[64, 8], [32, 2], [1, 16]])
                        nc.gpsimd.tensor_mul(
                            act, act, _ap(rc_t[c], 0, [[0, 8], [16, 2], [1, 16]]))
                        nc.gpsimd.tensor_add(act, act, t1)
                        # transpose 4 pair-blocks
                        tpo = 0 if which == "q" else 512
                        for i in range(4):
                            nc.tensor.transpose(
                                tp[:, tpo + i * 128: tpo + (i + 1) * 128],
                                qn[:, i * 128:(i + 1) * 128], eye_b)
                        dst = qT8 if which == "q" else kT8
                        dstap = _ap(dst, (half * 4) * TLOC + c * 128,
                                    [[TLOC, 4], [1, 128]])
                        if which == "q":
                            nc.scalar.copy(dstap, tp[:, tpo:tpo + 512])
                        else:
                            nc.vector.tensor_copy(dstap, tp[:, tpo:tpo + 512])

            # ---------------- phase B stages (per chunk c, pair p) ----------
            def stage_s(c, p):
                """mask+scores matmuls, merged exp, den, in-place normalize."""
                mask = mF if c == 1 else mR
                pt = PSA.tile([128, 2, 512], F32, tag="qkv")
                for hh in range(2):
                    off = p * TLOC
                    nc.tensor.matmul(
                        pt[:, hh, 0:256], eye_8, mask[:, :],
                        start=True, stop=False)
                    nc.tensor.matmul(
                        pt[:, hh, 0:256],
                        qT8[hh * 64:(hh + 1) * 64,
                            off + c * 128: off + (c + 1) * 128],
                        kT8[hh * 64:(hh + 1) * 64,
                            off + (c - 1) * 128: off + (c + 1) * 128],
                        start=False, stop=True)
                e_sb = PB.tile([128, 512], BF16, tag="e_sb")
                nc.scalar.activation(
                    out=_ap(e_sb, 0, [[256, 2], [1, 256]]),
                    in_=pt[:, :, 0:256], func=AF.Exp)
                den = PB.tile([128, 2], F32, tag="den")
                nc.vector.tensor_reduce(
                    out=den, in_=e_sb.rearrange("p (h k) -> p h k", h=2),
                    axis=mybir.AxisListType.X, op=ALU.add)
                invd = PB.tile([128, 2], F32, tag="invd")
                nc.vector.reciprocal(out=invd, in_=den)
                nc.vector.tensor_mul(
                    e_sb.rearrange("p (h k) -> p h k", h=2),
                    e_sb.rearrange("p (h k) -> p h k", h=2),
                    _ap(invd, 0, [[1, 2], [0, 256]]))
                return e_sb

            def stage_t(c, p, e_sb):
                """probs transpose into psum (bf16), copy to sbuf."""
                tp = PST.tile([128, 1024], BF16, tag="tp")
                for i in range(4):
                    nc.tensor.transpose(
                        tp[:, i * 128:(i + 1) * 128],
                        e_sb[:, i * 128:(i + 1) * 128], eye_b)
                pT = PB.tile([128, 512], BF16, tag="pT")
                if p % 2:
                    nc.scalar.copy(pT, tp[:, 0:512])
                else:
                    nc.vector.tensor_copy(pT, tp[:, 0:512])
                return pT

            def stage_v(c, p, pT, ugrp):
                """attn-out matmuls (bf16) + per-group fp8 store."""
                i = p % 4
                for hh in range(2):
                    for kc in range(2):
                        nc.tensor.matmul(
                            ugrp[hh * 64:(hh + 1) * 64, i * 128:(i + 1) * 128],
                            v_bf[:, (c - 1 + kc) * D + (2 * p + hh) * HD:
                                 (c - 1 + kc) * D + (2 * p + hh + 1) * HD],
                            pT[:, (hh * 2 + kc) * 128:(hh * 2 + kc + 1) * 128],
                            start=(kc == 0), stop=(kc == 1),
                            tile_position=(0, hh * 64))
                if i == 3:
                    grp = p // 4
                    nc.scalar.copy(
                        _ap(at8, (grp * 4) * 1024 + (c - 1) * 128,
                            [[1024, 4], [1, 128]]), ugrp)

            # ---------------- phase B+C for one chunk -----------------------
            def phase_bc(c):
                st_s = {}
                st_t = {}
                ugrp = [None]

                def get_ugrp(p):
                    if p % 4 == 0:
                        ugrp[0] = PSV.tile([128, 512], F32, tag="sv",
                                           name="ugrp")
                    return ugrp[0]

                for i in range(NPAIR + 3):
                    if i < NPAIR:
                        st_s[i] = stage_s(c, i)
                    if 2 <= i < NPAIR + 2:
                        st_t[i - 2] = stage_t(c, i - 2, st_s.pop(i - 2))
                    if i >= 3:
                        p = i - 3
                        stage_v(c, p, st_t.pop(p), get_ugrp(p))
                # o_proj + residual
                o_ps = PSA.tile([128, 2, 512], F32, tag="qkv")
                for half in range(2):
                    for kp in range(4):
                        nc.tensor.matmul(
                            o_ps[:, half, :],
                            _ap(at8, 2 * kp * 1024 + (c - 1) * 128,
                                [[1024, 2], [1, 128]]),
                            _ap(ow8, 2 * kp * D + half * 512,
                                [[D, 2], [1, 512]]),
                            start=(kp == 0), stop=(kp == 3), perf_mode=DR)
                for half in range(2):
                    yt = YP.tile([128, 512], F32, tag="y")
                    nc.vector.scalar_tensor_tensor(
                        out=yt, in0=o_ps[:, half, :], scalar=1.0 / 4096.0,
                        in1=x_sb[c][:, half * 512:(half + 1) * 512],
                        op0=ALU.mult, op1=ALU.add)
                    nc.sync.dma_start(
                        out=y[(c - 1) * 128:c * 128,
                              half * 512:(half + 1) * 512], in_=yt)

            # ---------------- interleaved schedule --------------------------
            phase_a(0)
            for c in range(1, NCH):
                phase_a(c)
                phase_bc(c)

    if waitfix:
        _split_excess_waits(nc)
    return nc


_PROGRAM = None


def _get_program():
    global _PROGRAM
    if _PROGRAM is None:
        _PROGRAM = build_program()
    return _PROGRAM


def _q8(a):
    return np.clip(a, -240.0, 240.0).astype(E4)


def _host_inputs(input_NTD, qkv_weight, o_weight, o_scale):
    x = np.asarray(input_NTD, dtype=np.float32)
    wq = np.asarray(qkv_weight, dtype=np.float32).reshape(3 * D, D)
    # [128, 8, 3D]: wT8[p, kt, j] = wq[j, kt*128+p] * 64
    wT8 = _q8(np.ascontiguousarray(
        (wq.T * 64.0).reshape(8, 128, 3 * D).transpose(1, 0, 2)))
    ows = np.asarray(o_weight, dtype=np.float32) * \
        np.asarray(o_scale, dtype=np.float32)[:, None]
    ow8 = _q8(np.ascontiguousarray(
        (ows.T * 4096.0).reshape(8, 128, D).transpose(1, 0, 2)))
    eyeb = np.eye(128, dtype=np.float32).astype(BF)
    eye8 = np.eye(128, dtype=np.float32).astype(E4)

    j = np.arange(W)[:, None]
    m = np.arange(2 * W)[None, :]
    base = (m > j) & (m <= W + j)
    maskR = np.where(base, 0.0, -240.0).astype(np.float32).astype(E4)
    maskF0 = np.where(base & (m >= W), 0.0, -240.0).astype(np.float32).astype(E4)

    freqs = (1.0 / 10000.0) ** np.linspace(0.0, 1.0, 16).astype(np.float32)

    in_maps = []
    for core in range(8):
        n, qq = divmod(core, 4)
        lo = qq * 1024 - 128
        if qq == 0:
            xs = np.concatenate(
                [np.zeros((128, D), np.float32), x[n, 0:1024]], axis=0)
        else:
            xs = x[n, lo:lo + 1024 + 128]
        xs = np.ascontiguousarray(xs)
        xT8 = _q8(np.ascontiguousarray(
            xs.T.reshape(8, 128, TLOC).transpose(1, 0, 2)))
        pos = np.maximum(np.arange(lo, lo + TLOC), 0).astype(np.float32)
        theta = pos[:, None] * freqs[None, :]
        cos16, sin16 = np.cos(theta), np.sin(theta)
        rotc = np.ascontiguousarray(np.concatenate(
            [cos16, cos16, sin16, -sin16], axis=1)).astype(BF)
        in_maps.append(dict(
            x_nat=xs, xT8=xT8, wT8=wT8, ow8=ow8, rotc=rotc,
            maskF=(maskF0 if qq == 0 else maskR), maskR=maskR,
            eyeb=eyeb, eye8=eye8))
    return in_maps


def kernel(input_NTD, qkv_weight, o_weight, o_scale, _trace=False):
    nc = _get_program()
    in_maps = _host_inputs(input_NTD, qkv_weight, o_weight, o_scale)
    res = run_bass_kernel_spmd(nc, in_maps, core_ids=list(range(8)),
                               trace=_trace)
    kernel.last_results = res
    out = np.empty((N, T, D), dtype=np.float32)
    for core in range(8):
        n, qq = divmod(core, 4)
        out[n, qq * 1024:(qq + 1) * 1024] = res.results[core]["y"]
    return out


# revision 13
# speedup vs baseline: 1.5066x; 1.1685x over previous
# Trainium2 Bass kernel: nn_DecoderAttentionLayer (sliding-window decoder layer)
# Sequence-parallel over 8 NeuronCores: core = (n, quarter); each core processes
# 1024 tokens (+128-token halo for the previous key/value chunk).
#
# v3 design notes:
#   - QKV and o_proj matmuls in fp8e4 DoubleRow mode (2 k-tiles per
#     instruction, 0.5 cycles/row) with weights pre-scaled on host
#     (w*64, ow*4096); scales folded into the rms/evac constants.
#   - q normalized on-chip before store (exp scale is the constant 1.0).
#   - mask applied ADDITIVELY pre-exp via eye@mask matmul accumulation.
#   - one merged EXP per pair (strided psum read), denominator via one DVE
#     reduce, probs normalized in-place on DVE before the PE transpose.
#   - attention probs/values kept bf16; attn output stored fp8 for the
#     DoubleRow o_proj.
#   - phases A (qkv+prep), B (attention), C (o_proj) interleaved per chunk;
#     per-pair software pipeline (S, T at -2, V at -3) keeps PE fed.
#   - x kept resident in SBUF for the residual; compact rotary table.
#   - all DMA issues on the sync queue (keeps compute-queue dispatch clean).
import sys
import numpy as np
import ml_dtypes

sys.path.insert(0, "/opt/trn_rl_repo")

import bass_rust
import concourse.bass as bass
import concourse.tile as tile
from concourse import mybir
from concourse.bass_utils import run_bass_kernel_spmd
from concourse.vector_clock import ScopedClock

F32 = mybir.dt.float32
BF16 = mybir.dt.bfloat16
F8 = mybir.dt.float8e4
AF = mybir.ActivationFunctionType
ALU = mybir.AluOpType
DR = mybir.MatmulPerfMode.DoubleRow
BF = ml_dtypes.bfloat16
E4 = ml_dtypes.float8_e4m3

N, T, D = 2, 4096, 1024
HD, NH, W = 64, 16, 128
EPS = 1.1920929e-07
TLOC = 1152          # 128 halo + 1024 own tokens
NCH = 9              # x chunks per core (chunk 0 = halo)
NPAIR = 8            # head pairs


# ---------------------------------------------------------------------------
# Compiler workarounds: walrus in this container accepts at most ONE sem wait
# per instruction on most structs. Split excess waits onto NoOps.
# ---------------------------------------------------------------------------
def _split_excess_waits(nc):
    cnt = 0
    for f in nc.m.functions:
        for b in f.blocks:
            changed = False
            new_insts = []
            for inst in b.instructions:
                si = inst.sync_info
                waits = list(si.on_wait) if (si is not None and si.on_wait) else []
                if len(waits) > 1:
                    si.on_wait = waits[:1]
                    for w in waits[1:]:
                        cnt += 1
                        nop = bass_rust.InstNoOp(
                            name=f"I-waitfix-{cnt}", engine=inst.engine)
                        nop.sync_info = mybir.SyncInfo(on_wait=[w], on_update=[])
                        new_insts.append(nop)
                    changed = True
                new_insts.append(inst)
            if changed:
                b.instructions = new_insts
    return cnt


def _patched_drain_and_barrier(self, tick_clock, wait_clock):
    drain_inst = self.nc.sync.drain()
    wait_clock.add_sem_waits(
        drain_inst.ins, ScopedClock({None: tick_clock.global_clock}))
    si = drain_inst.ins.sync_info
    if si is not None and si.on_wait and len(si.on_wait) > 1:
        waits = list(si.on_wait)
        si.on_wait = waits[:1]
        for w in waits[1:]:
            extra = self.nc.sync.drain()
            esi = extra.ins.sync_info
            if esi is None:
                extra.ins.sync_info = mybir.SyncInfo(on_wait=[w], on_update=[])
            else:
                esi.on_wait = [w]
    self.nc.all_engine_barrier()
    assert self.sems is not None
    popped = self.nc._tile_sem_poison_stack.pop()
    assert popped is self._sem_poison
    self.nc.clear_and_free_semaphores(list(self.sems.allocated().values()))
    self.nc.all_engine_barrier()


tile.TileContext._drain_and_barrier = _patched_drain_and_barrier


def _ap(t, offset, dims):
    return bass.AP(tensor=t.tensor, offset=t.offset + offset, ap=[t.ap[0]] + dims)


def build_program(waitfix=True):
    nc = bass.Bass()

    x_nat = nc.dram_tensor("x_nat", [TLOC, D], F32, kind="ExternalInput")
    xT8d = nc.dram_tensor("xT8", [128, 8, TLOC], F8, kind="ExternalInput")
    wT8d = nc.dram_tensor("wT8", [128, 8, 3 * D], F8, kind="ExternalInput")
    ow8d = nc.dram_tensor("ow8", [128, 8, D], F8, kind="ExternalInput")
    rotd = nc.dram_tensor("rotc", [TLOC, 64], BF16, kind="ExternalInput")
    mFd = nc.dram_tensor("maskF", [W, 2 * W], F8, kind="ExternalInput")
    mRd = nc.dram_tensor("maskR", [W, 2 * W], F8, kind="ExternalInput")
    eyebd = nc.dram_tensor("eyeb", [128, 128], BF16, kind="ExternalInput")
    eye8d = nc.dram_tensor("eye8", [128, 128], F8, kind="ExternalInput")
    y = nc.dram_tensor("y", [1024, D], F32, kind="ExternalOutput")

    with tile.TileContext(nc) as tc:
        with tc.tile_pool(name="persist", bufs=1) as P, \
             tc.tile_pool(name="qkwork", bufs=3) as QK, \
             tc.tile_pool(name="small", bufs=4) as SM, \
             tc.tile_pool(name="pbwork", bufs=4) as PB, \
             tc.tile_pool(name="ypool", bufs=2) as YP, \
             tc.tile_pool(name="ps_a", bufs=2, space="PSUM") as PSA, \
             tc.tile_pool(name="ps_v", bufs=2, space="PSUM") as PSV, \
             tc.tile_pool(name="ps_t", bufs=2, space="PSUM") as PST:

            # ---------------- persistent loads (order matters for startup) --
            x_sb = []
            for c in range(NCH):
                x_sb.append(P.tile([128, D], F32, tag=f"x{c}", name=f"x{c}"))
            xc8 = []
            for c in range(NCH):
                xc8.append(P.tile([128, 8, 128], F8, tag=f"xc{c}", name=f"xc{c}"))
            wT4 = []
            for kp in range(4):
                wT4.append(P.tile([128, 2, 3 * D], F8, tag=f"w{kp}", name=f"w{kp}"))
            rc_t = []
            for c in range(NCH):
                rc_t.append(P.tile([128, 64], BF16, tag=f"rc{c}", name=f"rc{c}"))

            nc.sync.dma_start(out=x_sb[0], in_=x_nat[0:128, :])
            nc.sync.dma_start(out=xc8[0], in_=xT8d[:, :, 0:128])
            for kp in range(4):
                for g in range(2):
                    nc.sync.dma_start(
                        out=wT4[kp][:, g, :],
                        in_=wT8d[:, 2 * kp + g, :])
            eye_b = P.tile([128, 128], BF16, tag="eyeb")
            nc.sync.dma_start(out=eye_b, in_=eyebd[:, :])
            eye_8 = P.tile([128, 128], F8, tag="eye8")
            nc.sync.dma_start(out=eye_8, in_=eye8d[:, :])
            mF = P.tile([W, 2 * W], F8, tag="mF")
            nc.sync.dma_start(out=mF, in_=mFd[:, :])
            mR = P.tile([W, 2 * W], F8, tag="mR")
            nc.sync.dma_start(out=mR, in_=mRd[:, :])
            nc.sync.dma_start(out=rc_t[0], in_=rotd[0:128, :])
            for c in range(1, NCH):
                nc.sync.dma_start(out=xc8[c], in_=xT8d[:, :, c * 128:(c + 1) * 128])
                nc.sync.dma_start(out=x_sb[c], in_=x_nat[c * 128:(c + 1) * 128, :])
                nc.sync.dma_start(out=rc_t[c], in_=rotd[c * 128:(c + 1) * 128, :])
            ow8 = P.tile([128, 8, D], F8, tag="ow8")
            nc.sync.dma_start(out=ow8, in_=ow8d[:, :, :])

            eps_t = P.tile([128, 1], F32, tag="eps")
            nc.vector.memset(eps_t, EPS)

            # persistent activation stores
            qT8 = P.tile([128, NPAIR * TLOC], BF16, tag="qT8")
            kT8 = P.tile([128, NPAIR * TLOC], BF16, tag="kT8")
            v_bf = P.tile([128, NCH * D], BF16, tag="v_bf")
            at8 = P.tile([128, NPAIR * 1024], F8, tag="at8")

            # ---------------- phase A for one chunk -------------------------
            phA_state = {}

            def phase_a_mm(c):
                # rms stats for x chunk
                bstats = SM.tile([128, 2, 6], F32, tag="bstats")
                for g in range(2):
                    nc.vector.bn_stats(out=bstats[:, g, :],
                                       in_=x_sb[c][:, g * 512:(g + 1) * 512])
                mv = SM.tile([128, 2], F32, tag="mv")
                nc.vector.bn_aggr(out=mv, in_=bstats)
                msq = SM.tile([128, 1], F32, tag="msq")
                nc.vector.tensor_mul(msq, mv[:, 0:1], mv[:, 0:1])
                nc.vector.tensor_add(msq, msq, mv[:, 1:2])
                rsq = SM.tile([128, 1], F32, tag="rsq")
                nc.scalar.activation(out=rsq, in_=msq, func=AF.Sqrt, bias=eps_t)
                inv = SM.tile([128, 1], F32, tag="inv")
                nc.vector.reciprocal(out=inv, in_=rsq)
                inv64 = SM.tile([128, 1], F32, tag="inv64")
                nc.vector.tensor_scalar_mul(out=inv64, in0=inv, scalar1=1.0 / 64.0)
                inv2 = SM.tile([128, 1], F32, tag="inv2")
                nc.vector.tensor_mul(inv2, inv, inv)

                for half in range(2):
                    ps = PSA.tile([128, 2, 512], F32, tag="qkv")
                    for kp in range(4):
                        lhs = _ap(xc8[c], 2 * kp * 128, [[128, 2], [1, 128]])
                        for slot, jlo in ((0, 0), (1, D)):
                            nc.tensor.matmul(
                                ps[:, slot, :], lhs,
                                _ap(wT4[kp], jlo + half * 512,
                                    [[3 * D, 2], [1, 512]]),
                                start=(kp == 0), stop=(kp == 3), perf_mode=DR)
                    vps = PSV.tile([128, 512], F32, tag="sv")
                    for kp in range(4):
                        nc.tensor.matmul(
                            vps, _ap(xc8[c], 2 * kp * 128, [[128, 2], [1, 128]]),
                            _ap(wT4[kp], 2 * D + half * 512,
                                [[3 * D, 2], [1, 512]]),
                            start=(kp == 0), stop=(kp == 3), perf_mode=DR)
                    # V evac: bf16 with inv/64 folded
                    nc.scalar.activation(
                        out=v_bf[:, c * D + half * 512: c * D + (half + 1) * 512],
                        in_=vps, func=AF.Copy, scale=inv64)
                    # q & k evac merged: stats, scale, rotary over all 16 heads
                    sq = QK.tile([128, 1024], BF16, tag="sq")
                    nc.scalar.activation(
                        out=sq.rearrange("p (s f) -> p s f", s=2),
                        in_=ps[:, :, :], func=AF.Square)
                    ssq = SM.tile([128, 16], F32, tag="ssq")
                    nc.vector.tensor_reduce(
                        out=ssq, in_=sq.rearrange("p (h d) -> p h d", h=16),
                        axis=mybir.AxisListType.X, op=ALU.add)
                    mt = SM.tile([128, 16], F32, tag="mt")
                    nc.vector.tensor_scalar(
                        out=mt, in0=ssq, scalar1=inv2,
                        scalar2=1.0 / (4096.0 * 64.0),
                        op0=ALU.mult, op1=ALU.mult)
                    rs = SM.tile([128, 16], F32, tag="rs")
                    nc.scalar.activation(out=rs, in_=mt, func=AF.Sqrt,
                                         bias=eps_t)
                    rr = SM.tile([128, 16], F32, tag="rr")
                    nc.vector.reciprocal(out=rr, in_=rs)
                    # q scale has extra 1/8; apply 1/64 to both, then fix q
                    scl = SM.tile([128, 16], F32, tag="scl")
                    nc.vector.tensor_scalar(
                        out=scl, in0=rr, scalar1=inv, scalar2=1.0 / 64.0,
                        op0=ALU.mult, op1=ALU.mult)
                    nc.vector.tensor_scalar_mul(
                        out=scl[:, 0:8], in0=scl[:, 0:8], scalar1=0.125)
                    qn = QK.tile([128, 1024], BF16, tag="qn")
                    nc.vector.tensor_mul(
                        _ap(qn, 0, [[512, 2], [64, 8], [1, 64]]),
                        _ap(ps, 0, [[512, 2], [64, 8], [1, 64]]),
                        _ap(scl, 0, [[8, 2], [1, 8], [0, 64]]))
                    # rotary on the active 16-col blocks (q & k together)
                    t1 = QK.tile([128, 16, 2, 16], BF16, tag="t1")
                    nc.gpsimd.tensor_mul(
                        t1, _ap(qn, 32, [[64, 16], [-32, 2], [1, 16]]),
                        _ap(rc_t[c], 32, [[0, 16], [16, 2], [1, 16]]))
                    act = _ap(qn, 0, [[64, 16], [32, 2], [1, 16]])
                    nc.gpsimd.tensor_mul(
                        act, act, _ap(rc_t[c], 0, [[0, 16], [16, 2], [1, 16]]))
                    nc.gpsimd.tensor_add(act, act, t1)
                    phA_state[(c, half)] = qn

            def phase_a_tp(c):
                for half in range(2):
                    qn = phA_state.pop((c, half))
                    tp = PST.tile([128, 1024], BF16, tag="tp")
                    nq = 0 if c == 0 else 4
                    for i in range(nq):
                        nc.tensor.transpose(
                            tp[:, i * 128:(i + 1) * 128],
                            qn[:, i * 128:(i + 1) * 128], eye_b)
                    for i in range(4):
                        nc.tensor.transpose(
                            tp[:, 512 + i * 128: 512 + (i + 1) * 128],
                            qn[:, 512 + i * 128: 512 + (i + 1) * 128], eye_b)
                    if nq:
                        nc.scalar.copy(
                            _ap(qT8, (half * 4) * TLOC + c * 128,
                                [[TLOC, 4], [1, 128]]),
                            tp[:, 0:512])
                    nc.vector.tensor_copy(
                        _ap(kT8, (half * 4) * TLOC + c * 128,
                            [[TLOC, 4], [1, 128]]),
                        tp[:, 512:1024])

            # ------- phase B super-stages (chunk c, even pair p: p,p+1) -----
            def stage_ss(c, p):
                """2 pairs: mask+scores, exps, one den/recip, gpsimd norm."""
                mask = mF if c == 1 else mR
                e2 = PB.tile([128, 1024], BF16, tag="e2")
                for sub in range(2):
                    pp = p + sub
                    pt = PSA.tile([128, 2, 512], F32, tag="qkv")
                    for hh in range(2):
                        off = pp * TLOC
                        nc.tensor.matmul(
                            pt[:, hh, 0:256], eye_8, mask[:, :],
                            start=True, stop=False)
                        nc.tensor.matmul(
                            pt[:, hh, 0:256],
                            qT8[hh * 64:(hh + 1) * 64,
                                off + c * 128: off + (c + 1) * 128],
                            kT8[hh * 64:(hh + 1) * 64,
                                off + (c - 1) * 128: off + (c + 1) * 128],
                            start=False, stop=True)
                    nc.scalar.activation(
                        out=_ap(e2, sub * 512, [[256, 2], [1, 256]]),
                        in_=pt[:, :, 0:256], func=AF.Exp)
                den = PB.tile([128, 4], F32, tag="den")
                nc.vector.tensor_reduce(
                    out=den, in_=e2.rearrange("p (h k) -> p h k", h=4),
                    axis=mybir.AxisListType.X, op=ALU.add)
                invd = PB.tile([128, 4], F32, tag="invd")
                nc.vector.reciprocal(out=invd, in_=den)
                nc.gpsimd.tensor_mul(
                    e2.rearrange("p (h k) -> p h k", h=4),
                    e2.rearrange("p (h k) -> p h k", h=4),
                    _ap(invd, 0, [[1, 4], [0, 256]]))
                return e2

            def stage_tt(c, p, e2):
                """2 pairs: 8 transposes into one psum tile, one copy out."""
                tp = PST.tile([128, 1024], BF16, tag="tp")
                for i in range(8):
                    nc.tensor.transpose(
                        tp[:, i * 128:(i + 1) * 128],
                        e2[:, i * 128:(i + 1) * 128], eye_b)
                pT = PB.tile([128, 1024], BF16, tag="pT")
                if p % 4:
                    nc.scalar.copy(pT, tp)
                else:
                    nc.vector.tensor_copy(pT, tp)
                return pT

            def stage_vv(c, p, pT, ugrp):
                """2 pairs: attn-out matmuls (bf16) + per-group fp8 store."""
                for sub in range(2):
                    pp = p + sub
                    i = pp % 4
                    for hh in range(2):
                        for kc in range(2):
                            nc.tensor.matmul(
                                ugrp[hh * 64:(hh + 1) * 64,
                                     i * 128:(i + 1) * 128],
                                v_bf[:, (c - 1 + kc) * D + (2 * pp + hh) * HD:
                                     (c - 1 + kc) * D + (2 * pp + hh + 1) * HD],
                                pT[:, (sub * 4 + hh * 2 + kc) * 128:
                                   (sub * 4 + hh * 2 + kc + 1) * 128],
                                start=(kc == 0), stop=(kc == 1),
                                tile_position=(0, hh * 64))
                if p % 4 == 2:
                    grp = p // 4
                    nc.scalar.copy(
                        _ap(at8, (grp * 4) * 1024 + (c - 1) * 128,
                            [[1024, 4], [1, 128]]), ugrp)

            # ---------------- phase B pipeline + phase C --------------------
            bc_state = {}

            def phase_b(c):
                st_s = {}
                st_t = {}
                ugrp = [None]

                def get_ugrp(p):
                    if p % 4 == 0:
                        ugrp[0] = PSV.tile([128, 512], F32, tag="sv",
                                           name="ugrp")
                    return ugrp[0]

                for s in range(6):
                    p = 2 * s
                    if s < 4:
                        st_s[s] = stage_ss(c, p)
                    if 1 <= s < 5:
                        st_t[s - 1] = stage_tt(c, 2 * (s - 1),
                                               st_s.pop(s - 1))
                    if s >= 2:
                        stage_vv(c, 2 * (s - 2), st_t.pop(s - 2),
                                 get_ugrp(2 * (s - 2)))

            def phase_c(c):
                o_ps = PSA.tile([128, 2, 512], F32, tag="qkv")
                for half in range(2):
                    for kp in range(4):
                        nc.tensor.matmul(
                            o_ps[:, half, :],
                            _ap(at8, 2 * kp * 1024 + (c - 1) * 128,
                                [[1024, 2], [1, 128]]),
                            _ap(ow8, 2 * kp * D + half * 512,
                                [[D, 2], [1, 512]]),
                            start=(kp == 0), stop=(kp == 3), perf_mode=DR)
                for half in range(2):
                    yt = YP.tile([128, 512], F32, tag="y")
                    nc.vector.scalar_tensor_tensor(
                        out=yt, in0=o_ps[:, half, :], scalar=1.0 / 4096.0,
                        in1=x_sb[c][:, half * 512:(half + 1) * 512],
                        op0=ALU.mult, op1=ALU.add)
                    nc.sync.dma_start(
                        out=y[(c - 1) * 128:c * 128,
                              half * 512:(half + 1) * 512], in_=yt)

            # ------------- chunk-level software-pipelined schedule ----------
            # A_mm(c) fills the PE while the previous chunk's B tail and the
            # current A evac chains drain; A_tp(c) runs once evacs are done.
            phase_a_mm(0)
            phase_a_mm(1)
            phase_a_tp(0)
            phase_a_tp(1)
            for c in range(1, NCH):
                phase_b(c)
                if c + 1 < NCH:
                    phase_a_mm(c + 1)
                phase_c(c)
                if c + 1 < NCH:
                    phase_a_tp(c + 1)
    if waitfix:
        _split_excess_waits(nc)
    return nc


_PROGRAM = None


def _get_program():
    global _PROGRAM
    if _PROGRAM is None:
        _PROGRAM = build_program()
    return _PROGRAM


def _q8(a):
    return np.clip(a, -240.0, 240.0).astype(E4)


def _host_inputs(input_NTD, qkv_weight, o_weight, o_scale):
    x = np.asarray(input_NTD, dtype=np.float32)
    wq = np.asarray(qkv_weight, dtype=np.float32).reshape(3 * D, D)
    # [128, 8, 3D]: wT8[p, kt, j] = wq[j, kt*128+p] * 64
    wT8 = _q8(np.ascontiguousarray(
        (wq.T * 64.0).reshape(8, 128, 3 * D).transpose(1, 0, 2)))
    ows = np.asarray(o_weight, dtype=np.float32) * \
        np.asarray(o_scale, dtype=np.float32)[:, None]
    ow8 = _q8(np.ascontiguousarray(
        (ows.T * 4096.0).reshape(8, 128, D).transpose(1, 0, 2)))
    eyeb = np.eye(128, dtype=np.float32).astype(BF)
    eye8 = np.eye(128, dtype=np.float32).astype(E4)

    j = np.arange(W)[:, None]
    m = np.arange(2 * W)[None, :]
    base = (m > j) & (m <= W + j)
    maskR = np.where(base, 0.0, -240.0).astype(np.float32).astype(E4)
    maskF0 = np.where(base & (m >= W), 0.0, -240.0).astype(np.float32).astype(E4)

    freqs = (1.0 / 10000.0) ** np.linspace(0.0, 1.0, 16).astype(np.float32)

    in_maps = []
    for core in range(8):
        n, qq = divmod(core, 4)
        lo = qq * 1024 - 128
        if qq == 0:
            xs = np.concatenate(
                [np.zeros((128, D), np.float32), x[n, 0:1024]], axis=0)
        else:
            xs = x[n, lo:lo + 1024 + 128]
        xs = np.ascontiguousarray(xs)
        xT8 = _q8(np.ascontiguousarray(
            xs.T.reshape(8, 128, TLOC).transpose(1, 0, 2)))
        pos = np.maximum(np.arange(lo, lo + TLOC), 0).astype(np.float32)
        theta = pos[:, None] * freqs[None, :]
        cos16, sin16 = np.cos(theta), np.sin(theta)
        rotc = np.ascontiguousarray(np.concatenate(
            [cos16, cos16, sin16, -sin16], axis=1)).astype(BF)
        in_maps.append(dict(
            x_nat=xs, xT8=xT8, wT8=wT8, ow8=ow8, rotc=rotc,
            maskF=(maskF0 if qq == 0 else maskR), maskR=maskR,
            eyeb=eyeb, eye8=eye8))
    return in_maps


def kernel(input_NTD, qkv_weight, o_weight, o_scale, _trace=False):
    nc = _get_program()
    in_maps = _host_inputs(input_NTD, qkv_weight, o_weight, o_scale)
    res = run_bass_kernel_spmd(nc, in_maps, core_ids=list(range(8)),
                               trace=_trace)
    kernel.last_results = res
    out = np.empty((N, T, D), dtype=np.float32)
    for core in range(8):
        n, qq = divmod(core, 4)
        out[n, qq * 1024:(qq + 1) * 1024] = res.results[core]["y"]
    return out


# revision 17
# speedup vs baseline: 1.5470x; 1.0268x over previous
# Trainium2 Bass kernel: nn_DecoderAttentionLayer (sliding-window decoder layer)
# Sequence-parallel over 8 NeuronCores: core = (n, quarter); each core processes
# 1024 tokens (+128-token halo for the previous key/value chunk).
#
# v3 design notes:
#   - QKV and o_proj matmuls in fp8e4 DoubleRow mode (2 k-tiles per
#     instruction, 0.5 cycles/row) with weights pre-scaled on host
#     (w*64, ow*4096); scales folded into the rms/evac constants.
#   - q normalized on-chip before store (exp scale is the constant 1.0).
#   - mask applied ADDITIVELY pre-exp via eye@mask matmul accumulation.
#   - one merged EXP per pair (strided psum read), denominator via one DVE
#     reduce, probs normalized in-place on DVE before the PE transpose.
#   - attention probs/values kept bf16; attn output stored fp8 for the
#     DoubleRow o_proj.
#   - phases A (qkv+prep), B (attention), C (o_proj) interleaved per chunk;
#     per-pair software pipeline (S, T at -2, V at -3) keeps PE fed.
#   - x kept resident in SBUF for the residual; compact rotary table.
#   - all DMA issues on the sync queue (keeps compute-queue dispatch clean).
import sys
import numpy as np
import ml_dtypes

sys.path.insert(0, "/opt/trn_rl_repo")

import bass_rust
import concourse.bass as bass
import concourse.tile as tile
from concourse import mybir
from concourse.bass_utils import run_bass_kernel_spmd
from concourse.vector_clock import ScopedClock

F32 = mybir.dt.float32
BF16 = mybir.dt.bfloat16
F8 = mybir.dt.float8e4
AF = mybir.ActivationFunctionType
ALU = mybir.AluOpType
DR = mybir.MatmulPerfMode.DoubleRow
BF = ml_dtypes.bfloat16
E4 = ml_dtypes.float8_e4m3

N, T, D = 2, 4096, 1024
HD, NH, W = 64, 16, 128
EPS = 1.1920929e-07
TLOC = 1152          # 128 halo + 1024 own tokens
NCH = 9              # x chunks per core (chunk 0 = halo)
NPAIR = 8            # head pairs


# ---------------------------------------------------------------------------
# Compiler workarounds: walrus in this container accepts at most ONE sem wait
# per instruction on most structs. Split excess waits onto NoOps.
# ---------------------------------------------------------------------------
def _split_excess_waits(nc):
    cnt = 0
    for f in nc.m.functions:
        for b in f.blocks:
            changed = False
            new_insts = []
            for inst in b.instructions:
                si = inst.sync_info
                waits = list(si.on_wait) if (si is not None and si.on_wait) else []
                if len(waits) > 1:
                    si.on_wait = waits[:1]
                    for w in waits[1:]:
                        cnt += 1
                        nop = bass_rust.InstNoOp(
                            name=f"I-waitfix-{cnt}", engine=inst.engine)
                        nop.sync_info = mybir.SyncInfo(on_wait=[w], on_update=[])
                        new_insts.append(nop)
                    changed = True
                new_insts.append(inst)
            if changed:
                b.instructions = new_insts
    return cnt


def _patched_drain_and_barrier(self, tick_clock, wait_clock):
    drain_inst = self.nc.sync.drain()
    wait_clock.add_sem_waits(
        drain_inst.ins, ScopedClock({None: tick_clock.global_clock}))
    si = drain_inst.ins.sync_info
    if si is not None and si.on_wait and len(si.on_wait) > 1:
        waits = list(si.on_wait)
        si.on_wait = waits[:1]
        for w in waits[1:]:
            extra = self.nc.sync.drain()
            esi = extra.ins.sync_info
            if esi is None:
                extra.ins.sync_info = mybir.SyncInfo(on_wait=[w], on_update=[])
            else:
                esi.on_wait = [w]
    self.nc.all_engine_barrier()
    assert self.sems is not None
    popped = self.nc._tile_sem_poison_stack.pop()
    assert popped is self._sem_poison
    self.nc.clear_and_free_semaphores(list(self.sems.allocated().values()))
    self.nc.all_engine_barrier()


tile.TileContext._drain_and_barrier = _patched_drain_and_barrier


def _ap(t, offset, dims):
    return bass.AP(tensor=t.tensor, offset=t.offset + offset, ap=[t.ap[0]] + dims)


def build_program(waitfix=True):
    nc = bass.Bass()

    x_nat = nc.dram_tensor("x_nat", [TLOC, D], BF16, kind="ExternalInput")
    xT8d = nc.dram_tensor("xT8", [128, 8, TLOC], F8, kind="ExternalInput")
    wT8d = nc.dram_tensor("wT8", [128, 8, 3 * D], F8, kind="ExternalInput")
    ow8d = nc.dram_tensor("ow8", [128, 8, D], F8, kind="ExternalInput")
    rotd = nc.dram_tensor("rotc", [TLOC, 64], BF16, kind="ExternalInput")
    mFd = nc.dram_tensor("maskF", [W, 4 * W], F8, kind="ExternalInput")
    mRd = nc.dram_tensor("maskR", [W, 4 * W], F8, kind="ExternalInput")
    eyebd = nc.dram_tensor("eyeb", [128, 128], BF16, kind="ExternalInput")
    eye8d = nc.dram_tensor("eye8", [128, 128], F8, kind="ExternalInput")
    y = nc.dram_tensor("y", [1024, D], F32, kind="ExternalOutput")

    with tile.TileContext(nc) as tc:
        with tc.tile_pool(name="persist", bufs=1) as P, \
             tc.tile_pool(name="qkwork", bufs=3) as QK, \
             tc.tile_pool(name="small", bufs=4) as SM, \
             tc.tile_pool(name="pbwork", bufs=3) as PB, \
             tc.tile_pool(name="ypool", bufs=2) as YP, \
             tc.tile_pool(name="ps_a", bufs=2, space="PSUM") as PSA, \
             tc.tile_pool(name="ps_v", bufs=3, space="PSUM") as PSV, \
             tc.tile_pool(name="ps_t", bufs=1, space="PSUM") as PST:

            # ---------------- persistent loads (order matters for startup) --
            x_sb = []
            for c in range(NCH):
                x_sb.append(P.tile([128, D], BF16, tag=f"x{c}", name=f"x{c}"))
            xc8 = []
            for c in range(NCH):
                xc8.append(P.tile([128, 8, 128], F8, tag=f"xc{c}", name=f"xc{c}"))
            wT4 = []
            for kp in range(4):
                wT4.append(P.tile([128, 2, 3 * D], F8, tag=f"w{kp}", name=f"w{kp}"))
            rc_t = []
            for c in range(NCH):
                rc_t.append(P.tile([128, 64], BF16, tag=f"rc{c}", name=f"rc{c}"))

            nc.sync.dma_start(out=x_sb[0], in_=x_nat[0:128, :])
            nc.sync.dma_start(out=xc8[0], in_=xT8d[:, :, 0:128])
            for cc in range(3):
                for kp in range(4):
                    for g in range(2):
                        nc.sync.dma_start(
                            out=wT4[kp][:, g, cc * D:(cc + 1) * D],
                            in_=wT8d[:, 2 * kp + g, cc * D:(cc + 1) * D])
            eye_b = P.tile([128, 128], BF16, tag="eyeb")
            nc.sync.dma_start(out=eye_b, in_=eyebd[:, :])
            eye_8 = P.tile([128, 128], F8, tag="eye8")
            nc.sync.dma_start(out=eye_8, in_=eye8d[:, :])
            mF = P.tile([W, 4 * W], F8, tag="mF")
            nc.sync.dma_start(out=mF, in_=mFd[:, :])
            mR = P.tile([W, 4 * W], F8, tag="mR")
            nc.sync.dma_start(out=mR, in_=mRd[:, :])
            nc.sync.dma_start(out=rc_t[0], in_=rotd[0:128, :])
            for c in range(1, NCH):
                nc.sync.dma_start(out=xc8[c], in_=xT8d[:, :, c * 128:(c + 1) * 128])
                nc.sync.dma_start(out=x_sb[c], in_=x_nat[c * 128:(c + 1) * 128, :])
                nc.sync.dma_start(out=rc_t[c], in_=rotd[c * 128:(c + 1) * 128, :])
            ow8 = P.tile([128, 8, D], F8, tag="ow8")
            nc.sync.dma_start(out=ow8, in_=ow8d[:, :, :])

            eps_t = P.tile([128, 1], F32, tag="eps")
            nc.vector.memset(eps_t, EPS)

            # persistent activation stores
            qTh = [P.tile([64, NPAIR * TLOC], BF16, tag=f"qT{h}", name=f"qT{h}")
                   for h in range(2)]
            kTh = [P.tile([64, NPAIR * TLOC], BF16, tag=f"kT{h}", name=f"kT{h}")
                   for h in range(2)]
            v_bf = P.tile([128, NCH * D], BF16, tag="v_bf")
            at8 = P.tile([128, NPAIR * 1024], F8, tag="at8")

            # ---------------- phase A for one chunk -------------------------
            phA_state = {}

            def phase_a_mm(c):
                # rms stats for x chunk
                bstats = SM.tile([128, 2, 6], F32, tag="bstats")
                for g in range(2):
                    nc.vector.bn_stats(out=bstats[:, g, :],
                                       in_=x_sb[c][:, g * 512:(g + 1) * 512])
                mv = SM.tile([128, 2], F32, tag="mv")
                nc.vector.bn_aggr(out=mv, in_=bstats)
                msq = SM.tile([128, 1], F32, tag="msq")
                nc.vector.tensor_mul(msq, mv[:, 0:1], mv[:, 0:1])
                nc.vector.tensor_add(msq, msq, mv[:, 1:2])
                rsq = SM.tile([128, 1], F32, tag="rsq")
                nc.scalar.activation(out=rsq, in_=msq, func=AF.Sqrt, bias=eps_t)
                inv = SM.tile([128, 1], F32, tag="inv")
                nc.vector.reciprocal(out=inv, in_=rsq)
                inv64 = SM.tile([128, 1], F32, tag="inv64")
                nc.vector.tensor_scalar_mul(out=inv64, in0=inv, scalar1=1.0 / 64.0)
                inv2 = SM.tile([128, 1], F32, tag="inv2")
                nc.vector.tensor_mul(inv2, inv, inv)

                for half in range(2):
                    ps = PSA.tile([128, 2, 512], F32, tag="qkv")
                    for kp in range(4):
                        lhs = _ap(xc8[c], 2 * kp * 128, [[128, 2], [1, 128]])
                        for slot, jlo in ((0, 0), (1, D)):
                            nc.tensor.matmul(
                                ps[:, slot, :], lhs,
                                _ap(wT4[kp], jlo + half * 512,
                                    [[3 * D, 2], [1, 512]]),
                                start=(kp == 0), stop=(kp == 3), perf_mode=DR)
                    vps = PSV.tile([128, 512], F32, tag="sv")
                    for kp in range(4):
                        nc.tensor.matmul(
                            vps, _ap(xc8[c], 2 * kp * 128, [[128, 2], [1, 128]]),
                            _ap(wT4[kp], 2 * D + half * 512,
                                [[3 * D, 2], [1, 512]]),
                            start=(kp == 0), stop=(kp == 3), perf_mode=DR)
                    # V evac: bf16 with inv/64 folded
                    nc.scalar.activation(
                        out=v_bf[:, c * D + half * 512: c * D + (half + 1) * 512],
                        in_=vps, func=AF.Copy, scale=inv64)
                    # q & k evac merged: stats, scale, rotary over all 16 heads
                    sq = QK.tile([128, 1024], BF16, tag="sq")
                    nc.scalar.activation(
                        out=sq.rearrange("p (s f) -> p s f", s=2),
                        in_=ps[:, :, :], func=AF.Square)
                    ssq = SM.tile([128, 16], F32, tag="ssq")
                    nc.vector.tensor_reduce(
                        out=ssq, in_=sq.rearrange("p (h d) -> p h d", h=16),
                        axis=mybir.AxisListType.X, op=ALU.add)
                    mt = SM.tile([128, 16], F32, tag="mt")
                    nc.vector.tensor_scalar(
                        out=mt, in0=ssq, scalar1=inv2,
                        scalar2=1.0 / (4096.0 * 64.0),
                        op0=ALU.mult, op1=ALU.mult)
                    rs = SM.tile([128, 16], F32, tag="rs")
                    nc.scalar.activation(out=rs, in_=mt, func=AF.Sqrt,
                                         bias=eps_t)
                    rr = SM.tile([128, 16], F32, tag="rr")
                    nc.vector.reciprocal(out=rr, in_=rs)
                    # q scale has extra 1/8; apply 1/64 to both, then fix q
                    scl = SM.tile([128, 16], F32, tag="scl")
                    nc.vector.tensor_scalar(
                        out=scl, in0=rr, scalar1=inv, scalar2=1.0 / 64.0,
                        op0=ALU.mult, op1=ALU.mult)
                    nc.vector.tensor_scalar_mul(
                        out=scl[:, 0:8], in0=scl[:, 0:8], scalar1=0.125)
                    qn = QK.tile([128, 1024], BF16, tag="qn")
                    nc.vector.tensor_mul(
                        _ap(qn, 0, [[512, 2], [64, 8], [1, 64]]),
                        _ap(ps, 0, [[512, 2], [64, 8], [1, 64]]),
                        _ap(scl, 0, [[8, 2], [1, 8], [0, 64]]))
                    # rotary on the active 16-col blocks (q & k together)
                    t1 = QK.tile([128, 16, 2, 16], BF16, tag="t1")
                    nc.vector.tensor_mul(
                        t1, _ap(qn, 32, [[64, 16], [-32, 2], [1, 16]]),
                        _ap(rc_t[c], 32, [[0, 16], [16, 2], [1, 16]]))
                    act = _ap(qn, 0, [[64, 16], [32, 2], [1, 16]])
                    nc.gpsimd.tensor_mul(
                        act, act, _ap(rc_t[c], 0, [[0, 16], [16, 2], [1, 16]]))
                    nc.gpsimd.tensor_add(act, act, t1)
                    phA_state[(c, half)] = qn

            def phase_a_tp(c):
                for half in range(2):
                    qn = phA_state.pop((c, half))
                    for which, base in (("q", 0), ("k", 512)):
                        if which == "q" and c == 0:
                            continue
                        tp = PST.tile([128, 1024], BF16, tag="tp")
                        for i in range(4):
                            for hh in range(2):
                                nc.tensor.transpose(
                                    tp[0:64, (2 * i + hh) * 128:
                                       (2 * i + hh + 1) * 128],
                                    qn[:, base + i * 128 + hh * 64:
                                       base + i * 128 + (hh + 1) * 64],
                                    eye_b)
                        dstt = qTh if which == "q" else kTh
                        for hh in range(2):
                            srcap = _ap(tp[0:64, :], hh * 128,
                                        [[256, 4], [1, 128]])
                            dstap = _ap(dstt[hh],
                                        (half * 4) * TLOC + c * 128,
                                        [[TLOC, 4], [1, 128]])
                            if which == "q":
                                nc.scalar.copy(dstap, srcap)
                            else:
                                nc.vector.tensor_copy(dstap, srcap)

            # ------- phase B super-stages (chunk c, even pair p: p,p+1) -----
            def stage_ss(c, p):
                """2 pairs: mask+scores, exps, one den/recip, DVE norm."""
                mask = mF if c == 1 else mR
                e2 = PB.tile([128, 1024], BF16, tag="e2")
                for sub in range(2):
                    pp = p + sub
                    pt = PSV.tile([128, 512], F32, tag="sv", name="s_ps")
                    off = pp * TLOC
                    nc.tensor.matmul(
                        pt[:, :], eye_8, mask[:, :],
                        start=True, stop=False)
                    for hh in range(2):
                        nc.tensor.matmul(
                            pt[:, hh * 256:(hh + 1) * 256],
                            qTh[hh][:, off + c * 128: off + (c + 1) * 128],
                            kTh[hh][:, off + (c - 1) * 128: off + (c + 1) * 128],
                            start=False, stop=(hh == 1), skip_group_check=True)
                    nc.scalar.activation(
                        out=e2[:, sub * 512:(sub + 1) * 512],
                        in_=pt[:, :], func=AF.Exp)
                den = PB.tile([128, 4], F32, tag="den")
                nc.vector.tensor_reduce(
                    out=den, in_=e2.rearrange("p (h k) -> p h k", h=4),
                    axis=mybir.AxisListType.X, op=ALU.add)
                invd = PB.tile([128, 4], F32, tag="invd")
                nc.vector.reciprocal(out=invd, in_=den)
                nc.vector.tensor_mul(
                    e2.rearrange("p (h k) -> p h k", h=4),
                    e2.rearrange("p (h k) -> p h k", h=4),
                    _ap(invd, 0, [[1, 4], [0, 256]]))
                return e2

            def stage_tt(c, p, e2):
                """2 pairs: 8 transposes into one psum tile, one copy out."""
                tp = PST.tile([128, 1024], BF16, tag="tp")
                for i in range(8):
                    nc.tensor.transpose(
                        tp[:, i * 128:(i + 1) * 128],
                        e2[:, i * 128:(i + 1) * 128], eye_b)
                pT = PB.tile([128, 1024], BF16, tag="pT")
                if p % 4:
                    nc.scalar.copy(pT, tp)
                else:
                    nc.vector.tensor_copy(pT, tp)
                return pT

            def stage_vv(c, p, pT, ugrp):
                """2 pairs: attn-out matmuls (bf16) + per-group fp8 store."""
                for sub in range(2):
                    pp = p + sub
                    i = pp % 4
                    for hh in range(2):
                        for kc in range(2):
                            nc.tensor.matmul(
                                ugrp[hh * 64:(hh + 1) * 64,
                                     i * 128:(i + 1) * 128],
                                v_bf[:, (c - 1 + kc) * D + (2 * pp + hh) * HD:
                                     (c - 1 + kc) * D + (2 * pp + hh + 1) * HD],
                                pT[:, (sub * 4 + hh * 2 + kc) * 128:
                                   (sub * 4 + hh * 2 + kc + 1) * 128],
                                start=(kc == 0), stop=(kc == 1),
                                tile_position=(0, hh * 64))
                if p % 4 == 2:
                    grp = p // 4
                    nc.scalar.copy(
                        _ap(at8, (grp * 4) * 1024 + (c - 1) * 128,
                            [[1024, 4], [1, 128]]), ugrp)

            # ---------------- phase B pipeline + phase C --------------------
            bc_state = {}

            def phase_b(c):
                st_s = {}
                st_t = {}
                ugrp = [None]

                def get_ugrp(p):
                    if p % 4 == 0:
                        ugrp[0] = PSV.tile([128, 512], F32, tag="sv",
                                           name="ugrp")
                    return ugrp[0]

                for s in range(6):
                    p = 2 * s
                    if s < 4:
                        st_s[s] = stage_ss(c, p)
                    if 1 <= s < 5:
                        st_t[s - 1] = stage_tt(c, 2 * (s - 1),
                                               st_s.pop(s - 1))
                    if s >= 2:
                        stage_vv(c, 2 * (s - 2), st_t.pop(s - 2),
                                 get_ugrp(2 * (s - 2)))

            def phase_c(c):
                o_ps = PSA.tile([128, 2, 512], F32, tag="qkv")
                for half in range(2):
                    for kp in range(4):
                        nc.tensor.matmul(
                            o_ps[:, half, :],
                            _ap(at8, 2 * kp * 1024 + (c - 1) * 128,
                                [[1024, 2], [1, 128]]),
                            _ap(ow8, 2 * kp * D + half * 512,
                                [[D, 2], [1, 512]]),
                            start=(kp == 0), stop=(kp == 3), perf_mode=DR)
                for half in range(2):
                    yt = YP.tile([128, 512], F32, tag="y")
                    nc.vector.scalar_tensor_tensor(
                        out=yt, in0=o_ps[:, half, :], scalar=1.0 / 4096.0,
                        in1=x_sb[c][:, half * 512:(half + 1) * 512],
                        op0=ALU.mult, op1=ALU.add)
                    nc.sync.dma_start(
                        out=y[(c - 1) * 128:c * 128,
                              half * 512:(half + 1) * 512], in_=yt)

            # ------------- chunk-level software-pipelined schedule ----------
            # A_mm(c) fills the PE while the previous chunk's B tail and the
            # current A evac chains drain; A_tp(c) runs once evacs are done.
            phase_a_mm(0)
            phase_a_mm(1)
            phase_a_tp(0)
            phase_a_tp(1)
            for c in range(1, NCH):
                phase_b(c)
                if c + 1 < NCH:
                    phase_a_mm(c + 1)
                phase_c(c)
                if c + 1 < NCH:
                    phase_a_tp(c + 1)
    if waitfix:
        _split_excess_waits(nc)
    return nc


_PROGRAM = None


def _get_program():
    global _PROGRAM
    if _PROGRAM is None:
        _PROGRAM = build_program()
    return _PROGRAM


def _q8(a):
    return np.clip(a, -240.0, 240.0).astype(E4)


def _host_inputs(input_NTD, qkv_weight, o_weight, o_scale):
    x = np.asarray(input_NTD, dtype=np.float32)
    wq = np.asarray(qkv_weight, dtype=np.float32).reshape(3 * D, D)
    # [128, 8, 3D]: wT8[p, kt, j] = wq[j, kt*128+p] * 64
    wT8 = _q8(np.ascontiguousarray(
        (wq.T * 64.0).reshape(8, 128, 3 * D).transpose(1, 0, 2)))
    ows = np.asarray(o_weight, dtype=np.float32) * \
        np.asarray(o_scale, dtype=np.float32)[:, None]
    ow8 = _q8(np.ascontiguousarray(
        (ows.T * 4096.0).reshape(8, 128, D).transpose(1, 0, 2)))
    eyeb = np.eye(128, dtype=np.float32).astype(BF)
    eye8 = np.eye(128, dtype=np.float32).astype(E4)

    j = np.arange(W)[:, None]
    m = np.arange(2 * W)[None, :]
    base = (m > j) & (m <= W + j)
    mR1 = np.where(base, 0.0, -240.0).astype(np.float32)
    mF1 = np.where(base & (m >= W), 0.0, -240.0).astype(np.float32)
    maskR = np.concatenate([mR1, mR1], axis=1).astype(E4)
    maskF0 = np.concatenate([mF1, mF1], axis=1).astype(E4)

    freqs = (1.0 / 10000.0) ** np.linspace(0.0, 1.0, 16).astype(np.float32)

    in_maps = []
    for core in range(8):
        n, qq = divmod(core, 4)
        lo = qq * 1024 - 128
        if qq == 0:
            xs = np.concatenate(
                [np.zeros((128, D), np.float32), x[n, 0:1024]], axis=0)
        else:
            xs = x[n, lo:lo + 1024 + 128]
        xs = np.ascontiguousarray(xs)
        xT8 = _q8(np.ascontiguousarray(
            xs.T.reshape(8, 128, TLOC).transpose(1, 0, 2)))
        pos = np.maximum(np.arange(lo, lo + TLOC), 0).astype(np.float32)
        theta = pos[:, None] * freqs[None, :]
        cos16, sin16 = np.cos(theta), np.sin(theta)
        rotc = np.ascontiguousarray(np.concatenate(
            [cos16, cos16, sin16, -sin16], axis=1)).astype(BF)
        in_maps.append(dict(
            x_nat=xs.astype(BF), xT8=xT8, wT8=wT8, ow8=ow8, rotc=rotc,
            maskF=(maskF0 if qq == 0 else maskR), maskR=maskR,
            eyeb=eyeb, eye8=eye8))
    return in_maps


def kernel(input_NTD, qkv_weight, o_weight, o_scale, _trace=False):
    nc = _get_program()
    in_maps = _host_inputs(input_NTD, qkv_weight, o_weight, o_scale)
    res = run_bass_kernel_spmd(nc, in_maps, core_ids=list(range(8)),
                               trace=_trace)
    kernel.last_results = res
    out = np.empty((N, T, D), dtype=np.float32)
    for core in range(8):
        n, qq = divmod(core, 4)
        out[n, qq * 1024:(qq + 1) * 1024] = res.results[core]["y"]
    return out


# revision 19
# speedup vs baseline: 1.5599x; 1.0083x over previous
# Trainium2 Bass kernel: nn_DecoderAttentionLayer (sliding-window decoder layer)
# Sequence-parallel over 8 NeuronCores: core = (n, quarter); each core processes
# 1024 tokens (+128-token halo for the previous key/value chunk).
#
# v3 design notes:
#   - QKV and o_proj matmuls in fp8e4 DoubleRow mode (2 k-tiles per
#     instruction, 0.5 cycles/row) with weights pre-scaled on host
#     (w*64, ow*4096); scales folded into the rms/evac constants.
#   - q normalized on-chip before store (exp scale is the constant 1.0).
#   - mask applied ADDITIVELY pre-exp via eye@mask matmul accumulation.
#   - one merged EXP per pair (strided psum read), denominator via one DVE
#     reduce, probs normalized in-place on DVE before the PE transpose.
#   - attention probs/values kept bf16; attn output stored fp8 for the
#     DoubleRow o_proj.
#   - phases A (qkv+prep), B (attention), C (o_proj) interleaved per chunk;
#     per-pair software pipeline (S, T at -2, V at -3) keeps PE fed.
#   - x kept resident in SBUF for the residual; compact rotary table.
#   - all DMA issues on the sync queue (keeps compute-queue dispatch clean).
import sys
import numpy as np
import ml_dtypes

sys.path.insert(0, "/opt/trn_rl_repo")

import bass_rust
import concourse.bass as bass
import concourse.tile as tile
from concourse import mybir
from concourse.bass_utils import run_bass_kernel_spmd
from concourse.vector_clock import ScopedClock

F32 = mybir.dt.float32
BF16 = mybir.dt.bfloat16
F8 = mybir.dt.float8e4
AF = mybir.ActivationFunctionType
ALU = mybir.AluOpType
DR = mybir.MatmulPerfMode.DoubleRow
BF = ml_dtypes.bfloat16
E4 = ml_dtypes.float8_e4m3

N, T, D = 2, 4096, 1024
HD, NH, W = 64, 16, 128
EPS = 1.1920929e-07
TLOC = 1152          # 128 halo + 1024 own tokens
NCH = 9              # x chunks per core (chunk 0 = halo)
NPAIR = 8            # head pairs


# ---------------------------------------------------------------------------
# Compiler workarounds: walrus in this container accepts at most ONE sem wait
# per instruction on most structs. Split excess waits onto NoOps.
# ---------------------------------------------------------------------------
def _split_excess_waits(nc):
    cnt = 0
    for f in nc.m.functions:
        for b in f.blocks:
            changed = False
            new_insts = []
            for inst in b.instructions:
                si = inst.sync_info
                waits = list(si.on_wait) if (si is not None and si.on_wait) else []
                if len(waits) > 1:
                    si.on_wait = waits[:1]
                    for w in waits[1:]:
                        cnt += 1
                        nop = bass_rust.InstNoOp(
                            name=f"I-waitfix-{cnt}", engine=inst.engine)
                        nop.sync_info = mybir.SyncInfo(on_wait=[w], on_update=[])
                        new_insts.append(nop)
                    changed = True
                new_insts.append(inst)
            if changed:
                b.instructions = new_insts
    return cnt


def _patched_drain_and_barrier(self, tick_clock, wait_clock):
    drain_inst = self.nc.sync.drain()
    wait_clock.add_sem_waits(
        drain_inst.ins, ScopedClock({None: tick_clock.global_clock}))
    si = drain_inst.ins.sync_info
    if si is not None and si.on_wait and len(si.on_wait) > 1:
        waits = list(si.on_wait)
        si.on_wait = waits[:1]
        for w in waits[1:]:
            extra = self.nc.sync.drain()
            esi = extra.ins.sync_info
            if esi is None:
                extra.ins.sync_info = mybir.SyncInfo(on_wait=[w], on_update=[])
            else:
                esi.on_wait = [w]
    self.nc.all_engine_barrier()
    assert self.sems is not None
    popped = self.nc._tile_sem_poison_stack.pop()
    assert popped is self._sem_poison
    self.nc.clear_and_free_semaphores(list(self.sems.allocated().values()))
    self.nc.all_engine_barrier()


tile.TileContext._drain_and_barrier = _patched_drain_and_barrier


def _ap(t, offset, dims):
    return bass.AP(tensor=t.tensor, offset=t.offset + offset, ap=[t.ap[0]] + dims)


def build_program(waitfix=True):
    nc = bass.Bass()

    x_nat = nc.dram_tensor("x_nat", [TLOC, D], BF16, kind="ExternalInput")
    xT8d = nc.dram_tensor("xT8", [128, 8, TLOC], F8, kind="ExternalInput")
    wT8d = nc.dram_tensor("wT8", [128, 8, 3 * D], F8, kind="ExternalInput")
    ow8d = nc.dram_tensor("ow8", [128, 8, D], F8, kind="ExternalInput")
    rotd = nc.dram_tensor("rotc", [TLOC, 64], BF16, kind="ExternalInput")
    mFd = nc.dram_tensor("maskF", [W, 4 * W], F8, kind="ExternalInput")
    mRd = nc.dram_tensor("maskR", [W, 4 * W], F8, kind="ExternalInput")
    eyebd = nc.dram_tensor("eyeb", [128, 128], BF16, kind="ExternalInput")
    eye8d = nc.dram_tensor("eye8", [128, 128], F8, kind="ExternalInput")
    y = nc.dram_tensor("y", [1024, D], F32, kind="ExternalOutput")

    with tile.TileContext(nc) as tc:
        with tc.tile_pool(name="persist", bufs=1) as P, \
             tc.tile_pool(name="qkwork", bufs=3) as QK, \
             tc.tile_pool(name="small", bufs=4) as SM, \
             tc.tile_pool(name="pbwork", bufs=3) as PB, \
             tc.tile_pool(name="ypool", bufs=2) as YP, \
             tc.tile_pool(name="ps_a", bufs=2, space="PSUM") as PSA, \
             tc.tile_pool(name="ps_v", bufs=3, space="PSUM") as PSV, \
             tc.tile_pool(name="ps_t", bufs=1, space="PSUM") as PST:

            # ---------------- persistent loads (order matters for startup) --
            x_sb = []
            for c in range(NCH):
                x_sb.append(P.tile([128, D], BF16, tag=f"x{c}", name=f"x{c}"))
            xc8 = []
            for c in range(NCH):
                xc8.append(P.tile([128, 8, 128], F8, tag=f"xc{c}", name=f"xc{c}"))
            wT4 = []
            for kp in range(4):
                wT4.append(P.tile([128, 2, 3 * D], F8, tag=f"w{kp}", name=f"w{kp}"))
            rc_t = []
            for c in range(NCH):
                rc_t.append(P.tile([128, 64], BF16, tag=f"rc{c}", name=f"rc{c}"))

            nc.sync.dma_start(out=x_sb[0], in_=x_nat[0:128, :])
            nc.sync.dma_start(out=xc8[0], in_=xT8d[:, :, 0:128])
            for cc in range(3):
                for kp in range(4):
                    for g in range(2):
                        nc.sync.dma_start(
                            out=wT4[kp][:, g, cc * D:(cc + 1) * D],
                            in_=wT8d[:, 2 * kp + g, cc * D:(cc + 1) * D])
            eye_b = P.tile([128, 128], BF16, tag="eyeb")
            nc.sync.dma_start(out=eye_b, in_=eyebd[:, :])
            eye_8 = P.tile([128, 128], F8, tag="eye8")
            nc.sync.dma_start(out=eye_8, in_=eye8d[:, :])
            mF = P.tile([W, 4 * W], F8, tag="mF")
            nc.sync.dma_start(out=mF, in_=mFd[:, :])
            mR = P.tile([W, 4 * W], F8, tag="mR")
            nc.sync.dma_start(out=mR, in_=mRd[:, :])
            nc.sync.dma_start(out=rc_t[0], in_=rotd[0:128, :])
            for c in range(1, NCH):
                nc.sync.dma_start(out=xc8[c], in_=xT8d[:, :, c * 128:(c + 1) * 128])
                nc.sync.dma_start(out=x_sb[c], in_=x_nat[c * 128:(c + 1) * 128, :])
                nc.sync.dma_start(out=rc_t[c], in_=rotd[c * 128:(c + 1) * 128, :])
            ow8 = P.tile([128, 8, D], F8, tag="ow8")
            nc.sync.dma_start(out=ow8, in_=ow8d[:, :, :])

            eps_t = P.tile([128, 1], F32, tag="eps")
            nc.vector.memset(eps_t, EPS)

            # persistent activation stores
            qT2 = P.tile([64, 2 * NPAIR * TLOC], BF16, tag="qT2")
            kT2 = P.tile([64, 2 * NPAIR * TLOC], BF16, tag="kT2")
            v_bf = P.tile([128, NCH * D], BF16, tag="v_bf")
            at8 = P.tile([128, NPAIR * 1024], F8, tag="at8")

            # ---------------- phase A for one chunk -------------------------
            phA_state = {}

            def phase_a_mm(c):
                # rms stats for x chunk
                bstats = SM.tile([128, 2, 6], F32, tag="bstats")
                for g in range(2):
                    nc.vector.bn_stats(out=bstats[:, g, :],
                                       in_=x_sb[c][:, g * 512:(g + 1) * 512])
                mv = SM.tile([128, 2], F32, tag="mv")
                nc.vector.bn_aggr(out=mv, in_=bstats)
                msq = SM.tile([128, 1], F32, tag="msq")
                nc.vector.tensor_mul(msq, mv[:, 0:1], mv[:, 0:1])
                nc.vector.tensor_add(msq, msq, mv[:, 1:2])
                rsq = SM.tile([128, 1], F32, tag="rsq")
                nc.scalar.activation(out=rsq, in_=msq, func=AF.Sqrt, bias=eps_t)
                inv = SM.tile([128, 1], F32, tag="inv")
                nc.vector.reciprocal(out=inv, in_=rsq)
                inv64 = SM.tile([128, 1], F32, tag="inv64")
                nc.vector.tensor_scalar_mul(out=inv64, in0=inv, scalar1=1.0 / 64.0)
                inv2 = SM.tile([128, 1], F32, tag="inv2")
                nc.vector.tensor_mul(inv2, inv, inv)

                for half in range(2):
                    ps = PSA.tile([128, 2, 512], F32, tag="qkv")
                    for kp in range(4):
                        lhs = _ap(xc8[c], 2 * kp * 128, [[128, 2], [1, 128]])
                        for slot, jlo in ((0, 0), (1, D)):
                            nc.tensor.matmul(
                                ps[:, slot, :], lhs,
                                _ap(wT4[kp], jlo + half * 512,
                                    [[3 * D, 2], [1, 512]]),
                                start=(kp == 0), stop=(kp == 3), perf_mode=DR)
                    vps = PSV.tile([128, 512], F32, tag="sv")
                    for kp in range(4):
                        nc.tensor.matmul(
                            vps, _ap(xc8[c], 2 * kp * 128, [[128, 2], [1, 128]]),
                            _ap(wT4[kp], 2 * D + half * 512,
                                [[3 * D, 2], [1, 512]]),
                            start=(kp == 0), stop=(kp == 3), perf_mode=DR)
                    # V evac: bf16 with inv/64 folded
                    nc.scalar.activation(
                        out=v_bf[:, c * D + half * 512: c * D + (half + 1) * 512],
                        in_=vps, func=AF.Copy, scale=inv64)
                    # q & k evac merged: stats, scale, rotary over all 16 heads
                    sq = QK.tile([128, 1024], BF16, tag="sq")
                    nc.scalar.activation(
                        out=sq.rearrange("p (s f) -> p s f", s=2),
                        in_=ps[:, :, :], func=AF.Square)
                    ssq = SM.tile([128, 16], F32, tag="ssq")
                    nc.vector.tensor_reduce(
                        out=ssq, in_=sq.rearrange("p (h d) -> p h d", h=16),
                        axis=mybir.AxisListType.X, op=ALU.add)
                    mt = SM.tile([128, 16], F32, tag="mt")
                    nc.vector.tensor_scalar(
                        out=mt, in0=ssq, scalar1=inv2,
                        scalar2=1.0 / (4096.0 * 64.0),
                        op0=ALU.mult, op1=ALU.mult)
                    rs = SM.tile([128, 16], F32, tag="rs")
                    nc.scalar.activation(out=rs, in_=mt, func=AF.Sqrt,
                                         bias=eps_t)
                    rr = SM.tile([128, 16], F32, tag="rr")
                    nc.vector.reciprocal(out=rr, in_=rs)
                    # q scale has extra 1/8; apply 1/64 to both, then fix q
                    scl = SM.tile([128, 16], F32, tag="scl")
                    nc.vector.tensor_scalar(
                        out=scl, in0=rr, scalar1=inv, scalar2=1.0 / 64.0,
                        op0=ALU.mult, op1=ALU.mult)
                    nc.vector.tensor_scalar_mul(
                        out=scl[:, 0:8], in0=scl[:, 0:8], scalar1=0.125)
                    qn = QK.tile([128, 1024], BF16, tag="qn")
                    nc.vector.tensor_mul(
                        _ap(qn, 0, [[512, 2], [64, 8], [1, 64]]),
                        _ap(ps, 0, [[512, 2], [64, 8], [1, 64]]),
                        _ap(scl, 0, [[8, 2], [1, 8], [0, 64]]))
                    # rotary on the active 16-col blocks (q & k together)
                    t1 = QK.tile([128, 16, 2, 16], BF16, tag="t1")
                    nc.vector.tensor_mul(
                        t1, _ap(qn, 32, [[64, 16], [-32, 2], [1, 16]]),
                        _ap(rc_t[c], 32, [[0, 16], [16, 2], [1, 16]]))
                    act = _ap(qn, 0, [[64, 16], [32, 2], [1, 16]])
                    nc.gpsimd.tensor_mul(
                        act, act, _ap(rc_t[c], 0, [[0, 16], [16, 2], [1, 16]]))
                    nc.gpsimd.tensor_add(act, act, t1)
                    phA_state[(c, half)] = qn

            def phase_a_tp(c):
                for half in range(2):
                    qn = phA_state.pop((c, half))
                    for which, base in (("q", 0), ("k", 512)):
                        if which == "q" and c == 0:
                            continue
                        tp = PST.tile([128, 1024], BF16, tag="tp")
                        for i in range(4):
                            for hh in range(2):
                                nc.tensor.transpose(
                                    tp[0:64, (2 * i + hh) * 128:
                                       (2 * i + hh + 1) * 128],
                                    qn[:, base + i * 128 + hh * 64:
                                       base + i * 128 + (hh + 1) * 64],
                                    eye_b)
                        dstt = qT2 if which == "q" else kT2
                        srcap = _ap(tp[0:64, :], 0,
                                    [[256, 4], [128, 2], [1, 128]])
                        dstap = _ap(dstt, (half * 4) * TLOC + c * 128,
                                    [[TLOC, 4], [NPAIR * TLOC, 2], [1, 128]])
                        if which == "q":
                            nc.scalar.copy(dstap, srcap)
                        else:
                            nc.vector.tensor_copy(dstap, srcap)

            # ------- phase B super-stages (chunk c, even pair p: p,p+1) -----
            def stage_ss(c, p):
                """2 pairs: mask+scores, exps, one den/recip, DVE norm."""
                mask = mF if c == 1 else mR
                e2 = PB.tile([128, 1024], BF16, tag="e2")
                for sub in range(2):
                    pp = p + sub
                    pt = PSV.tile([128, 512], F32, tag="sv", name="s_ps")
                    off = pp * TLOC
                    nc.tensor.matmul(
                        pt[:, :], eye_8, mask[:, :],
                        start=True, stop=False)
                    for hh in range(2):
                        hb = hh * NPAIR * TLOC
                        nc.tensor.matmul(
                            pt[:, hh * 256:(hh + 1) * 256],
                            qT2[:, hb + off + c * 128: hb + off + (c + 1) * 128],
                            kT2[:, hb + off + (c - 1) * 128:
                                hb + off + (c + 1) * 128],
                            start=False, stop=True, skip_group_check=True)
                    nc.scalar.activation(
                        out=e2[:, sub * 512:(sub + 1) * 512],
                        in_=pt[:, :], func=AF.Exp)
                den = PB.tile([128, 4], F32, tag="den")
                nc.vector.tensor_reduce(
                    out=den, in_=e2.rearrange("p (h k) -> p h k", h=4),
                    axis=mybir.AxisListType.X, op=ALU.add)
                invd = PB.tile([128, 4], F32, tag="invd")
                nc.vector.reciprocal(out=invd, in_=den)
                nc.gpsimd.tensor_mul(
                    e2.rearrange("p (h k) -> p h k", h=4),
                    e2.rearrange("p (h k) -> p h k", h=4),
                    _ap(invd, 0, [[1, 4], [0, 256]]))
                return e2

            def stage_tt(c, p, e2):
                """2 pairs: 8 transposes into one psum tile, one copy out."""
                tp = PST.tile([128, 1024], BF16, tag="tp")
                for i in range(8):
                    nc.tensor.transpose(
                        tp[:, i * 128:(i + 1) * 128],
                        e2[:, i * 128:(i + 1) * 128], eye_b)
                pT = PB.tile([128, 1024], BF16, tag="pT")
                if p % 4:
                    nc.scalar.copy(pT, tp)
                else:
                    nc.vector.tensor_copy(pT, tp)
                return pT

            def stage_vv(c, p, pT, ugrp):
                """2 pairs: attn-out matmuls (bf16) + per-group fp8 store."""
                for sub in range(2):
                    pp = p + sub
                    i = pp % 4
                    for hh in range(2):
                        for kc in range(2):
                            nc.tensor.matmul(
                                ugrp[hh * 64:(hh + 1) * 64,
                                     i * 128:(i + 1) * 128],
                                v_bf[:, (c - 1 + kc) * D + (2 * pp + hh) * HD:
                                     (c - 1 + kc) * D + (2 * pp + hh + 1) * HD],
                                pT[:, (sub * 4 + hh * 2 + kc) * 128:
                                   (sub * 4 + hh * 2 + kc + 1) * 128],
                                start=(kc == 0), stop=(kc == 1),
                                tile_position=(0, hh * 64))
                if p % 4 == 2:
                    grp = p // 4
                    nc.scalar.copy(
                        _ap(at8, (grp * 4) * 1024 + (c - 1) * 128,
                            [[1024, 4], [1, 128]]), ugrp)

            # ---------------- phase B pipeline + phase C --------------------
            bc_state = {}

            def phase_b(c):
                st_s = {}
                st_t = {}
                ugrp = [None]

                def get_ugrp(p):
                    if p % 4 == 0:
                        ugrp[0] = PSV.tile([128, 512], F32, tag="sv",
                                           name="ugrp")
                    return ugrp[0]

                for s in range(6):
                    p = 2 * s
                    if s < 4:
                        st_s[s] = stage_ss(c, p)
                    if 1 <= s < 5:
                        st_t[s - 1] = stage_tt(c, 2 * (s - 1),
                                               st_s.pop(s - 1))
                    if s >= 2:
                        stage_vv(c, 2 * (s - 2), st_t.pop(s - 2),
                                 get_ugrp(2 * (s - 2)))

            def phase_c(c):
                o_ps = PSA.tile([128, 2, 512], F32, tag="qkv")
                for half in range(2):
                    for kp in range(4):
                        nc.tensor.matmul(
                            o_ps[:, half, :],
                            _ap(at8, 2 * kp * 1024 + (c - 1) * 128,
                                [[1024, 2], [1, 128]]),
                            _ap(ow8, 2 * kp * D + half * 512,
                                [[D, 2], [1, 512]]),
                            start=(kp == 0), stop=(kp == 3), perf_mode=DR)
                for half in range(2):
                    yt = YP.tile([128, 512], F32, tag="y")
                    nc.vector.scalar_tensor_tensor(
                        out=yt, in0=o_ps[:, half, :], scalar=1.0 / 4096.0,
                        in1=x_sb[c][:, half * 512:(half + 1) * 512],
                        op0=ALU.mult, op1=ALU.add)
                    nc.sync.dma_start(
                        out=y[(c - 1) * 128:c * 128,
                              half * 512:(half + 1) * 512], in_=yt)

            # ------------- chunk-level software-pipelined schedule ----------
            # A_mm(c) fills the PE while the previous chunk's B tail and the
            # current A evac chains drain; A_tp(c) runs once evacs are done.
            phase_a_mm(0)
            phase_a_mm(1)
            phase_a_tp(0)
            phase_a_tp(1)
            for c in range(1, NCH):
                phase_b(c)
                if c + 1 < NCH:
                    phase_a_mm(c + 1)
                phase_c(c)
                if c + 1 < NCH:
                    phase_a_tp(c + 1)
    if waitfix:
        _split_excess_waits(nc)
    return nc


_PROGRAM = None


def _get_program():
    global _PROGRAM
    if _PROGRAM is None:
        _PROGRAM = build_program()
    return _PROGRAM


def _q8(a):
    return np.clip(a, -240.0, 240.0).astype(E4)


def _host_inputs(input_NTD, qkv_weight, o_weight, o_scale):
    x = np.asarray(input_NTD, dtype=np.float32)
    wq = np.asarray(qkv_weight, dtype=np.float32).reshape(3 * D, D)
    # [128, 8, 3D]: wT8[p, kt, j] = wq[j, kt*128+p] * 64
    wT8 = _q8(np.ascontiguousarray(
        (wq.T * 64.0).reshape(8, 128, 3 * D).transpose(1, 0, 2)))
    ows = np.asarray(o_weight, dtype=np.float32) * \
        np.asarray(o_scale, dtype=np.float32)[:, None]
    ow8 = _q8(np.ascontiguousarray(
        (ows.T * 4096.0).reshape(8, 128, D).transpose(1, 0, 2)))
    eyeb = np.eye(128, dtype=np.float32).astype(BF)
    eye8 = np.eye(128, dtype=np.float32).astype(E4)

    j = np.arange(W)[:, None]
    m = np.arange(2 * W)[None, :]
    base = (m > j) & (m <= W + j)
    mR1 = np.where(base, 0.0, -240.0).astype(np.float32)
    mF1 = np.where(base & (m >= W), 0.0, -240.0).astype(np.float32)
    maskR = np.concatenate([mR1, mR1], axis=1).astype(E4)
    maskF0 = np.concatenate([mF1, mF1], axis=1).astype(E4)

    freqs = (1.0 / 10000.0) ** np.linspace(0.0, 1.0, 16).astype(np.float32)

    in_maps = []
    for core in range(8):
        n, qq = divmod(core, 4)
        lo = qq * 1024 - 128
        if qq == 0:
            xs = np.concatenate(
                [np.zeros((128, D), np.float32), x[n, 0:1024]], axis=0)
        else:
            xs = x[n, lo:lo + 1024 + 128]
        xs = np.ascontiguousarray(xs)
        xT8 = _q8(np.ascontiguousarray(
            xs.T.reshape(8, 128, TLOC).transpose(1, 0, 2)))
        pos = np.maximum(np.arange(lo, lo + TLOC), 0).astype(np.float32)
        theta = pos[:, None] * freqs[None, :]
        cos16, sin16 = np.cos(theta), np.sin(theta)
        rotc = np.ascontiguousarray(np.concatenate(
            [cos16, cos16, sin16, -sin16], axis=1)).astype(BF)
        in_maps.append(dict(
            x_nat=xs.astype(BF), xT8=xT8, wT8=wT8, ow8=ow8, rotc=rotc,
            maskF=(maskF0 if qq == 0 else maskR), maskR=maskR,
            eyeb=eyeb, eye8=eye8))
    return in_maps


def kernel(input_NTD, qkv_weight, o_weight, o_scale, _trace=False):
    nc = _get_program()
    in_maps = _host_inputs(input_NTD, qkv_weight, o_weight, o_scale)
    res = run_bass_kernel_spmd(nc, in_maps, core_ids=list(range(8)),
                               trace=_trace)
    kernel.last_results = res
    out = np.empty((N, T, D), dtype=np.float32)
    for core in range(8):
        n, qq = divmod(core, 4)
        out[n, qq * 1024:(qq + 1) * 1024] = res.results[core]["y"]
    return out
